# revision 18
# baseline (speedup 1.0000x reference)
"""AttnRCR Trainium2 kernel: 2-layer recurrent cross-attention (B=128, NQ=256, NC=128, D=1024).

Data-parallel over 8 NeuronCores (16 batch elements each). All matmuls in fp32r
(1 cyc/row on PE at N>=256, ~1e-4 accuracy); activations fp32 on ACT (single
`exp_and_others` table: Exp/Tanh/Square/Relu/Copy); rsqrt for the two l2norms
computed on DVE via bit-hack + 2 Newton iterations (no ACT table switch).

Layout strategy: keep activations transposed (feature dim on partitions) so every
matmul contraction lands on partitions without per-scan transposes:
  qT/ctxT (d,q)/(d,c) once per batch via PE transposes; sq=(q-wc)^2 produced
  directly in (d,q); MLP runs in transposed layout where per-feature biases are
  free per-partition ACT bias APs; softmax runs in (q,c) where the per-row smooth
  is a per-partition ACT scale AP.
"""
import numpy as np

B, NQ, NC, D = 128, 256, 128, 1024
AD, NL, SMOOTH0 = 256, 2, 10.0
NCORES = 8
NB = B // NCORES  # batches per core


def _build(n_b):
    import concourse.bacc as bacc
    import concourse.tile as tile
    import concourse.mybir as mybir
    from concourse import masks

    fp32 = mybir.dt.float32
    fp32r = mybir.dt.float32r
    i32 = mybir.dt.int32
    AF = mybir.ActivationFunctionType
    ALU = mybir.AluOpType

    nc = bacc.Bacc("TRN2", debug=False, enable_asserts=False, num_devices=NCORES)

    # ---- DRAM I/O (per-core shard: n_b batches; weights replicated) ----
    q_d = nc.dram_tensor("query", (n_b, NQ, D), fp32, kind="ExternalInput").ap()
    c_d = nc.dram_tensor("context", (n_b, NC, D), fp32, kind="ExternalInput").ap()
    cwW_d = nc.dram_tensor("cw_W", (NL, D, AD), fp32, kind="ExternalInput").ap()
    cwb_d = nc.dram_tensor("cw_b", (NL, AD), fp32, kind="ExternalInput").ap()
    sw1_d = nc.dram_tensor("sw_W1", (NL, AD, AD // 2), fp32, kind="ExternalInput").ap()
    sb1_d = nc.dram_tensor("sw_b1", (NL, AD // 2), fp32, kind="ExternalInput").ap()
    sw2_d = nc.dram_tensor("sw_W2", (NL, AD // 2, 1), fp32, kind="ExternalInput").ap()
    sb2_d = nc.dram_tensor("sw_b2", (NL, 1), fp32, kind="ExternalInput").ap()
    mw1_d = nc.dram_tensor("mw_W1", (NL, AD, 2 * AD), fp32, kind="ExternalInput").ap()
    mb1_d = nc.dram_tensor("mw_b1", (NL, 2 * AD), fp32, kind="ExternalInput").ap()
    mw2_d = nc.dram_tensor("mw_W2", (NL, 2 * AD, D), fp32, kind="ExternalInput").ap()
    mb2_d = nc.dram_tensor("mw_b2", (NL, D), fp32, kind="ExternalInput").ap()
    wc_d = nc.dram_tensor("wc_out", (n_b, NQ, D), fp32, kind="ExternalOutput").ap()
    at_d = nc.dram_tensor("attn_out", (n_b, NQ, NC), fp32, kind="ExternalOutput").ap()

    with tile.TileContext(nc) as tc:
        with tc.tile_pool(name="wp", bufs=1) as wp, \
             tc.tile_pool(name="sb", bufs=2) as sb, \
             tc.tile_pool(name="st3", bufs=3) as st3, \
             tc.tile_pool(name="sq9", bufs=8) as sq9, \
             tc.tile_pool(name="nr", bufs=4) as nrp, \
             tc.tile_pool(name="ps", bufs=4, space="PSUM") as psp:

            # ================= one-time: identity + weights (fp32r) =================
            id32 = wp.tile([128, 128], fp32, tag="id32")
            masks.make_identity(nc, id32[:])
            idr = wp.tile([128, 128], fp32r, tag="idr")
            nc.vector.tensor_copy(idr[:], id32[:])
            ones1 = wp.tile([1, 128], fp32, tag="ones1f")
            nc.vector.memset(ones1[:], 1.0)
            ones1r = wp.tile([1, 128], fp32r, tag="ones1")
            nc.vector.tensor_copy(ones1r[:], ones1[:])

            def load_conv(dst_ap, src_ap, shape, tag):
                st = sb.tile(shape, fp32, tag="qnat")
                nc.sync.dma_start(st[:], src_ap)
                nc.vector.tensor_copy(dst_ap, st[:])

            cw = wp.tile([128, NL, 8, AD], fp32r, tag="cw")
            mw1 = wp.tile([128, NL, 2, 2 * AD], fp32r, tag="mw1")
            mw2 = wp.tile([128, NL, 4, D], fp32r, tag="mw2")
            sw1 = wp.tile([128, NL, 2, AD // 2], fp32r, tag="sw1")
            sw2 = wp.tile([128, NL], fp32, tag="sw2")
            cwb = wp.tile([1, NL, AD], fp32r, tag="cwb")
            for i in range(NL):
                load_conv(cw[:, i], cwW_d[i].rearrange("(k p) a -> p k a", p=128),
                          [128, 8, AD], "wstg")
                load_conv(mw1[:, i], mw1_d[i].rearrange("(k p) m -> p k m", p=128),
                          [128, 2, 2 * AD], "wstg")
                for kk in range(4):
                    load_conv(mw2[:, i, kk], mw2_d[i, kk * 128:(kk + 1) * 128],
                              [128, D], "wstg")
                load_conv(sw1[:, i], sw1_d[i].rearrange("(k p) h -> p k h", p=128),
                          [128, 2, AD // 2], "wstg2")
            load_conv(cwb[:], cwb_d[None, :, :], [1, NL, AD], "wstg4")
            nc.sync.dma_start(sw2[:], sw2_d.rearrange("i p x -> p (i x)"))
            # fp32 per-partition bias tiles
            mb1 = wp.tile([128, NL, 4], fp32, tag="mb1")
            nc.sync.dma_start(mb1[:], mb1_d.rearrange("i (t p) -> p i t", p=128))
            mb2 = wp.tile([128, NL, 8], fp32, tag="mb2")
            nc.sync.dma_start(mb2[:], mb2_d.rearrange("i (t p) -> p i t", p=128))
            sb1 = wp.tile([128, NL], fp32, tag="sb1")
            nc.sync.dma_start(sb1[:], sb1_d.rearrange("i p -> p i"))
            # sw_b2: (NL,1) -> broadcast over partitions via stride-0 DMA read
            sb2 = wp.tile([128, NL], fp32, tag="sb2")
            nc.sync.dma_start(sb2[:], sb2_d.rearrange("i x -> (i x)")[None, :].broadcast_to((128, NL)))
            sb2p10 = wp.tile([128, 1], fp32, tag="sb2p10")
            nc.vector.tensor_single_scalar(sb2p10[:], sb2[:, 0:1], SMOOTH0, op=ALU.add)

            # ================= helpers =================
            def rsqrt(ssq, w, tag):
                """1/sqrt(ssq) on DVE, (128, w) fp32. Bit-hack seed + 2 Newton."""
                y = nrp.tile([128, w], fp32, tag=f"{tag}y")
                u = nrp.tile([128, w], fp32, tag=f"{tag}u")
                m = nrp.tile([128, w], fp32, tag=f"{tag}m")
                yi = y[:].bitcast(i32)
                nc.vector.tensor_single_scalar(yi, ssq.bitcast(i32), 1, op=ALU.arith_shift_right)
                nc.vector.tensor_scalar(yi, yi, -1, 0x5F3759DF, op0=ALU.mult, op1=ALU.add)
                nc.vector.tensor_single_scalar(m[:], ssq, -0.5, op=ALU.mult)
                for _ in range(2):
                    nc.vector.tensor_tensor(u[:], y[:], m[:], op=ALU.mult)
                    nc.vector.tensor_tensor(u[:], u[:], y[:], op=ALU.mult)
                    nc.vector.tensor_single_scalar(u[:], u[:], 1.5, op=ALU.add)
                    nc.vector.tensor_tensor(y[:], y[:], u[:], op=ALU.mult)
                return y

            # ================= per-batch body =================
            def batch(b):
                # ---- load + transpose query/context ----
                qT = st3.tile([128, 8, NQ], fp32r, tag="qT")
                for qt in range(2):
                    qn = sb.tile([128, D], fp32, tag="qnat")
                    nc.sync.dma_start(qn[:], q_d[b, qt * 128:(qt + 1) * 128])
                    for g in range(2):  # 4 dt per psum tile
                        tp = psp.tile([128, 512], fp32, tag="tr", bufs=2)
                        for j in range(4):
                            dt = g * 4 + j
                            nc.tensor.transpose(tp[:, j * 128:(j + 1) * 128],
                                                qn[:, dt * 128:(dt + 1) * 128], id32[:])
                        nc.vector.tensor_copy(
                            qT[:, g * 4:(g + 1) * 4, qt * 128:(qt + 1) * 128],
                            tp[:].rearrange("p (j q) -> p j q", j=4))
                ctxf = sb.tile([128, D], fp32, tag="ctxf")
                nc.sync.dma_start(ctxf[:], c_d[b])
                ctx = st3.tile([128, D], fp32r, tag="ctx")
                nc.gpsimd.tensor_copy(ctx[:], ctxf[:])
                ctxT = st3.tile([128, 8, NC], fp32r, tag="ctxT")
                for g in range(2):
                    tp = psp.tile([128, 512], fp32, tag="tr", bufs=2)
                    for j in range(4):
                        dt = g * 4 + j
                        nc.tensor.transpose(tp[:, j * 128:(j + 1) * 128],
                                            ctxf[:, dt * 128:(dt + 1) * 128], id32[:])
                    nc.scalar.copy(ctxT[:, g * 4:(g + 1) * 4],
                                   tp[:].rearrange("p (j c) -> p j c", j=4))

                mx = st3.tile([128, 8, NQ], fp32, tag="mx")        # matrix^T state
                smooth = st3.tile([128, 2], fp32, tag="smooth")    # per (q%128, qt)

                def scan(i, use_mx, final):
                    pl = psp.tile([128, 512], fp32, tag="ps")
                    if use_mx:
                        qps = []
                        for h in range(2):
                            qp4 = sb.tile([128, 4, NQ], fp32r, tag="qp4")
                            nc.vector.scalar_tensor_tensor(qp4[:], mx[:, h * 4:(h + 1) * 4], 1.0,
                                                           qT[:, h * 4:(h + 1) * 4],
                                                           op0=ALU.min, op1=ALU.mult)
                            qps.append(qp4)
                    for k in range(8):
                        rhs = qps[k // 4][:, k % 4] if use_mx else qT[:, k]
                        nc.tensor.matmul(pl[:, :NQ], ctxT[:, k], rhs,
                                         start=(k == 0), stop=(k == 7))
                    plc = sb.tile([128, NQ], fp32, tag="scr")
                    nc.scalar.copy(plc[:], pl[:, :NQ])
                    lr = sb.tile([128, NQ], fp32, tag="lr")
                    nc.vector.scalar_tensor_tensor(lr[:], plc[:], 0.1, plc[:],
                                                   op0=ALU.mult, op1=ALU.max)
                    scr = sb.tile([128, NQ], fp32, tag="scr")
                    ssq = nrp.tile([128, 1], fp32, tag="ssq")
                    nc.scalar.activation(scr[:], lr[:], AF.Square, accum_out=ssq[:])
                    rs = rsqrt(ssq[:], 1, "a")
                    lrn = sb.tile([128, NQ], fp32r, tag="lrn")
                    nc.vector.tensor_scalar_mul(lrn[:], lr[:], rs[:])
                    pt = psp.tile([128, 512], fp32r, tag="tr", bufs=2)
                    for qt in range(2):
                        nc.tensor.transpose(pt[:, qt * 128:(qt + 1) * 128],
                                            lrn[:, qt * 128:(qt + 1) * 128], idr[:])
                    e = sb.tile([128, 2, NC], fp32, tag="scr")
                    sume = nrp.tile([128, 2], fp32, tag="sume")
                    for qt in range(2):
                        sc = 10.0 if i == 0 else smooth[:, qt:qt + 1]
                        nc.scalar.activation(e[:, qt], pt[:, qt * 128:(qt + 1) * 128],
                                             AF.Exp, scale=sc, accum_out=sume[:, qt:qt + 1])
                    rse = nrp.tile([128, 2], fp32, tag="rse")
                    nc.vector.reciprocal(rse[:], sume[:])
                    p = sb.tile([128, 2, NC], fp32r, tag="p")
                    for qt in range(2):
                        nc.vector.tensor_scalar_mul(p[:, qt], e[:, qt], rse[:, qt:qt + 1])
                    pp = psp.tile([128, 512], fp32r, tag="tr", bufs=2)
                    for qt in range(2):
                        nc.tensor.transpose(pp[:, qt * 128:(qt + 1) * 128], p[:, qt], idr[:])
                    pT = sb.tile([128, NQ], fp32r, tag="pT")
                    nc.vector.tensor_copy(pT[:], pp[:, :NQ])
                    if final:
                        nc.sync.dma_start(at_d[b].rearrange("(t p) c -> p t c", p=128),
                                          p[:].bitcast(fp32))
                        for qt in range(2):
                            for nh in range(2):
                                pw = psp.tile([128, 512], fp32, tag="wc", bufs=2)
                                nc.tensor.matmul(pw[:], pT[:, qt * 128:(qt + 1) * 128],
                                                 ctx[:, nh * 512:(nh + 1) * 512],
                                                 start=True, stop=True)
                                wch = sb.tile([128, 512], fp32, tag="wch")
                                nc.scalar.copy(wch[:], pw[:])
                                nc.sync.dma_start(
                                    wc_d[b, qt * 128:(qt + 1) * 128,
                                         nh * 512:(nh + 1) * 512], wch[:])
                        return None
                    sqTs = []
                    for dt in range(8):
                        pw = psp.tile([128, 512], fp32, tag="wc", bufs=2)
                        nc.tensor.matmul(pw[:, :NQ], ctx[:, dt * 128:(dt + 1) * 128],
                                         pT[:], start=True, stop=True)
                        qmw = sb.tile([128, NQ], fp32, tag="scr")
                        nc.vector.tensor_tensor(qmw[:], qT[:, dt], pw[:, :NQ], op=ALU.subtract)
                        sq = sq9.tile([128, NQ], fp32r, tag="sqT")
                        nc.scalar.activation(sq[:], qmw[:], AF.Square)
                        sqTs.append(sq)
                    return sqTs

                def mlp(i, sqTs):
                    ssqc = nrp.tile([128, 2], fp32, tag="ssqc")
                    pcs = []
                    for _qt in range(2):
                        pc = psp.tile([128, 512], fp32, tag="ps")
                        pcs.append(pc)
                    for k in range(8):
                        for qt in range(2):
                            nc.tensor.matmul(pcs[qt][:, :AD],
                                             sqTs[k][:, qt * 128:(qt + 1) * 128],
                                             cw[:, i, k], start=(k == 0), stop=False)
                    for qt in range(2):
                        nc.tensor.matmul(pcs[qt][:, :AD], ones1r[:], cwb[:, i],
                                         start=False, stop=True)
                        scr2 = sb.tile([128, AD], fp32, tag="scr")
                        nc.scalar.activation(scr2[:], pcs[qt][:, :AD], AF.Square,
                                             accum_out=ssqc[:, qt:qt + 1])
                    rsc = rsqrt(ssqc[:], 2, "c")
                    cn = sb.tile([128, 2, AD], fp32r, tag="cn")
                    for qt in range(2):
                        nc.vector.tensor_scalar_mul(cn[:, qt], pcs[qt][:, :AD], rsc[:, qt:qt + 1])
                    pcT = psp.tile([128, 512], fp32r, tag="tr", bufs=2)
                    for ah in range(2):
                        for qt in range(2):
                            nc.tensor.transpose(pcT[:, ah * 256 + qt * 128:ah * 256 + qt * 128 + 128],
                                                cn[:, qt, ah * 128:(ah + 1) * 128], idr[:])
                    cnT = sb.tile([128, 2, NQ], fp32r, tag="cnT")
                    nc.scalar.copy(cnT[:], pcT[:].rearrange("p (a q) -> p a q", a=2))
                    # matrix branch
                    t1m = sb.tile([128, 4, NQ], fp32r, tag="t1m")
                    for mt in range(4):
                        pm = psp.tile([128, 512], fp32, tag="ps")
                        for ah in range(2):
                            nc.tensor.matmul(pm[:, :NQ], mw1[:, i, ah, mt * 128:(mt + 1) * 128],
                                             cnT[:, ah], start=(ah == 0), stop=(ah == 1))
                        nc.scalar.activation(t1m[:, mt], pm[:, :NQ], AF.Tanh,
                                             bias=mb1[:, i, mt:mt + 1])
                    for dt in range(8):
                        pm2 = psp.tile([128, 512], fp32, tag="ps")
                        for mt in range(4):
                            nc.tensor.matmul(pm2[:, :NQ], mw2[:, i, mt, dt * 128:(dt + 1) * 128],
                                             t1m[:, mt], start=(mt == 0), stop=(mt == 3))
                        th = sb.tile([128, NQ], fp32, tag="lr")
                        nc.scalar.activation(th[:], pm2[:, :NQ], AF.Tanh, bias=mb2[:, i, dt:dt + 1])
                        if i == 0:
                            # matrix stored UNCLIPPED; clip fused into consumers.
                            # clip == min(.,1): value >= -1 always (tanh>=-1, prev>=0)
                            nc.vector.tensor_single_scalar(mx[:, dt], th[:], 1.0, op=ALU.add)
                        else:
                            nc.vector.scalar_tensor_tensor(mx[:, dt], mx[:, dt], 1.0, th[:],
                                                           op0=ALU.min, op1=ALU.add)
                    # smooth branch
                    ps1 = psp.tile([128, 512], fp32, tag="ps")
                    for ah in range(2):
                        nc.tensor.matmul(ps1[:, :NQ], sw1[:, i, ah], cnT[:, ah],
                                         start=(ah == 0), stop=(ah == 1))
                    t1s = sb.tile([128, NQ], fp32, tag="t1s")
                    nc.scalar.activation(t1s[:], ps1[:, :NQ], AF.Tanh, bias=sb1[:, i:i + 1])
                    if i != 0:
                        sm2 = nrp.tile([128, 2], fp32, tag="sm2")
                        nc.vector.tensor_scalar_add(sm2[:], smooth[:], sb2[:, i:i + 1])
                    for qt in range(2):
                        pso = psp.tile([128, 512], fp32, tag="ps")
                        nc.tensor.matmul(pso[:, :1], t1s[:, qt * 128:(qt + 1) * 128],
                                         sw2[:, i:i + 1], start=True, stop=True)
                        ba = sb2p10[:] if i == 0 else sm2[:, qt:qt + 1]
                        nc.scalar.activation(smooth[:, qt:qt + 1], pso[:, :1], AF.Relu, bias=ba)

                # ---- the recurrence ----
                sq0 = scan(0, False, False)
                mlp(0, sq0)
                sq1 = scan(1, True, False)
                mlp(1, sq1)
                scan(2, True, True)

            for b in range(n_b):
                batch(b)

    nc.compile()
    return nc


_NC_CACHE = {}


def _get_nc(n_b):
    if n_b not in _NC_CACHE:
        _NC_CACHE[n_b] = _build(n_b)
    return _NC_CACHE[n_b]


def kernel(query, context, cw_W, cw_b, sw_W1, sw_b1, sw_W2, sw_b2,
           mw_W1, mw_b1, mw_W2, mw_b2, trace=False):
    from concourse import bass_utils

    query = np.ascontiguousarray(np.asarray(query, dtype=np.float32))
    context = np.ascontiguousarray(np.asarray(context, dtype=np.float32))
    w = {k: np.ascontiguousarray(np.asarray(v, dtype=np.float32)) for k, v in dict(
        cw_W=cw_W, cw_b=cw_b, sw_W1=sw_W1, sw_b1=sw_b1, sw_W2=sw_W2, sw_b2=sw_b2,
        mw_W1=mw_W1, mw_b1=mw_b1, mw_W2=mw_W2, mw_b2=mw_b2).items()}

    nc = _get_nc(NB)
    in_maps = []
    for c in range(NCORES):
        sl = slice(c * NB, (c + 1) * NB)
        in_maps.append({"query": query[sl], "context": context[sl], **w})
    res = bass_utils.run_bass_kernel_spmd(nc, in_maps, core_ids=list(range(NCORES)),
                                          trace=trace)
    wc = np.concatenate([res.results[c]["wc_out"] for c in range(NCORES)], axis=0)
    attn = np.concatenate([res.results[c]["attn_out"] for c in range(NCORES)], axis=0)
    kernel.last_results = res
    return query, wc, attn


kernel.last_results = None


# revision 24
# speedup vs baseline: 1.0631x; 1.0631x over previous
"""AttnRCR Trainium2 kernel: 2-layer recurrent cross-attention (B=128, NQ=256, NC=128, D=1024).

Data-parallel over 8 NeuronCores (16 batch elements each). All matmuls in fp32r
(1 cyc/row on PE at N>=256, ~1e-4 accuracy); activations fp32 on ACT (single
`exp_and_others` table: Exp/Tanh/Square/Relu/Copy); rsqrt for the two l2norms
computed on DVE via bit-hack + 2 Newton iterations (no ACT table switch).

Layout strategy: keep activations transposed (feature dim on partitions) so every
matmul contraction lands on partitions without per-scan transposes:
  qT/ctxT (d,q)/(d,c) once per batch via PE transposes; sq=(q-wc)^2 produced
  directly in (d,q); MLP runs in transposed layout where per-feature biases are
  free per-partition ACT bias APs; softmax runs in (q,c) where the per-row smooth
  is a per-partition ACT scale AP.
"""
import numpy as np

B, NQ, NC, D = 128, 256, 128, 1024
AD, NL, SMOOTH0 = 256, 2, 10.0
NCORES = 8
NB = B // NCORES  # batches per core


def _build(n_b):
    import concourse.bacc as bacc
    import concourse.tile as tile
    import concourse.mybir as mybir
    from concourse import masks

    fp32 = mybir.dt.float32
    fp32r = mybir.dt.float32r
    i32 = mybir.dt.int32
    AF = mybir.ActivationFunctionType
    ALU = mybir.AluOpType

    nc = bacc.Bacc("TRN2", debug=False, enable_asserts=False, num_devices=NCORES)

    # ---- DRAM I/O (per-core shard: n_b batches; weights replicated) ----
    q_d = nc.dram_tensor("query", (n_b, NQ, D), fp32, kind="ExternalInput").ap()
    c_d = nc.dram_tensor("context", (n_b, NC, D), fp32, kind="ExternalInput").ap()
    cwW_d = nc.dram_tensor("cw_W", (NL, D, AD), fp32, kind="ExternalInput").ap()
    cwb_d = nc.dram_tensor("cw_b", (NL, AD), fp32, kind="ExternalInput").ap()
    sw1_d = nc.dram_tensor("sw_W1", (NL, AD, AD // 2), fp32, kind="ExternalInput").ap()
    sb1_d = nc.dram_tensor("sw_b1", (NL, AD // 2), fp32, kind="ExternalInput").ap()
    sw2_d = nc.dram_tensor("sw_W2", (NL, AD // 2, 1), fp32, kind="ExternalInput").ap()
    sb2_d = nc.dram_tensor("sw_b2", (NL, 1), fp32, kind="ExternalInput").ap()
    mw1_d = nc.dram_tensor("mw_W1", (NL, AD, 2 * AD), fp32, kind="ExternalInput").ap()
    mb1_d = nc.dram_tensor("mw_b1", (NL, 2 * AD), fp32, kind="ExternalInput").ap()
    mw2_d = nc.dram_tensor("mw_W2", (NL, 2 * AD, D), fp32, kind="ExternalInput").ap()
    mb2_d = nc.dram_tensor("mw_b2", (NL, D), fp32, kind="ExternalInput").ap()
    wc_d = nc.dram_tensor("wc_out", (n_b, NQ, D), fp32, kind="ExternalOutput").ap()
    at_d = nc.dram_tensor("attn_out", (n_b, NQ, NC), fp32, kind="ExternalOutput").ap()

    with tile.TileContext(nc) as tc:
        with tc.tile_pool(name="wp", bufs=1) as wp, \
             tc.tile_pool(name="sb", bufs=2) as sb, \
             tc.tile_pool(name="st3", bufs=3) as st3, \
             tc.tile_pool(name="sq9", bufs=8) as sq9, \
             tc.tile_pool(name="nr", bufs=4) as nrp, \
             tc.tile_pool(name="ps", bufs=4, space="PSUM") as psp:

            # ================= one-time: identity + weights (fp32r) =================
            id32 = wp.tile([128, 128], fp32, tag="id32")
            masks.make_identity(nc, id32[:])
            idr = wp.tile([128, 128], fp32r, tag="idr")
            nc.vector.tensor_copy(idr[:], id32[:])
            ones1 = wp.tile([1, 128], fp32, tag="ones1f")
            nc.vector.memset(ones1[:], 1.0)
            ones1r = wp.tile([1, 128], fp32r, tag="ones1")
            nc.vector.tensor_copy(ones1r[:], ones1[:])

            def load_conv(dst_ap, src_ap, shape, tag):
                st = sb.tile(shape, fp32, tag="qnat")
                nc.sync.dma_start(st[:], src_ap)
                nc.vector.tensor_copy(dst_ap, st[:])

            cw = wp.tile([128, NL, 8, AD], fp32r, tag="cw")
            mw1 = wp.tile([128, NL, 2, 2 * AD], fp32r, tag="mw1")
            mw2 = wp.tile([128, NL, 4, D], fp32r, tag="mw2")
            sw1 = wp.tile([128, NL, 2, AD // 2], fp32r, tag="sw1")
            sw2 = wp.tile([128, NL], fp32, tag="sw2")
            cwb = wp.tile([1, NL, AD], fp32r, tag="cwb")
            for i in range(NL):
                load_conv(cw[:, i], cwW_d[i].rearrange("(k p) a -> p k a", p=128),
                          [128, 8, AD], "wstg")
                load_conv(mw1[:, i], mw1_d[i].rearrange("(k p) m -> p k m", p=128),
                          [128, 2, 2 * AD], "wstg")
                for kk in range(4):
                    load_conv(mw2[:, i, kk], mw2_d[i, kk * 128:(kk + 1) * 128],
                              [128, D], "wstg")
                load_conv(sw1[:, i], sw1_d[i].rearrange("(k p) h -> p k h", p=128),
                          [128, 2, AD // 2], "wstg2")
            load_conv(cwb[:], cwb_d[None, :, :], [1, NL, AD], "wstg4")
            nc.sync.dma_start(sw2[:], sw2_d.rearrange("i p x -> p (i x)"))
            # fp32 per-partition bias tiles
            mb1 = wp.tile([128, NL, 4], fp32, tag="mb1")
            nc.sync.dma_start(mb1[:], mb1_d.rearrange("i (t p) -> p i t", p=128))
            mb2 = wp.tile([128, NL, 8], fp32, tag="mb2")
            nc.sync.dma_start(mb2[:], mb2_d.rearrange("i (t p) -> p i t", p=128))
            sb1 = wp.tile([128, NL], fp32, tag="sb1")
            nc.sync.dma_start(sb1[:], sb1_d.rearrange("i p -> p i"))
            # sw_b2: (NL,1) -> broadcast over partitions via stride-0 DMA read
            sb2 = wp.tile([128, NL], fp32, tag="sb2")
            nc.sync.dma_start(sb2[:], sb2_d.rearrange("i x -> (i x)")[None, :].broadcast_to((128, NL)))
            sb2p10 = wp.tile([128, 1], fp32, tag="sb2p10")
            nc.vector.tensor_single_scalar(sb2p10[:], sb2[:, 0:1], SMOOTH0, op=ALU.add)

            # ================= helpers =================
            def rsqrt(ssq, w, tag):
                """1/sqrt(ssq) on DVE, (128, w) fp32. Bit-hack seed + 2 Newton."""
                y = nrp.tile([128, w], fp32, tag=f"{tag}y")
                u = nrp.tile([128, w], fp32, tag=f"{tag}u")
                m = nrp.tile([128, w], fp32, tag=f"{tag}m")
                yi = y[:].bitcast(i32)
                nc.vector.tensor_single_scalar(yi, ssq.bitcast(i32), 1, op=ALU.arith_shift_right)
                nc.vector.tensor_scalar(yi, yi, -1, 0x5F3759DF, op0=ALU.mult, op1=ALU.add)
                nc.vector.tensor_single_scalar(m[:], ssq, -0.5, op=ALU.mult)
                for _ in range(2):
                    nc.vector.tensor_tensor(u[:], y[:], m[:], op=ALU.mult)
                    nc.vector.tensor_tensor(u[:], u[:], y[:], op=ALU.mult)
                    nc.vector.tensor_single_scalar(u[:], u[:], 1.5, op=ALU.add)
                    nc.vector.tensor_tensor(y[:], y[:], u[:], op=ALU.mult)
                return y

            # ================= per-batch body =================
            def load(b):
                # ---- load + transpose query/context ----
                qT = st3.tile([128, 8, NQ], fp32r, tag="qT")
                for qt in range(2):
                    qn = sb.tile([128, D], fp32, tag="qnat")
                    nc.sync.dma_start(qn[:], q_d[b, qt * 128:(qt + 1) * 128])
                    for g in range(2):  # 4 dt per psum tile
                        tp = psp.tile([128, 512], fp32, tag="tr", bufs=2)
                        for j in range(4):
                            dt = g * 4 + j
                            nc.tensor.transpose(tp[:, j * 128:(j + 1) * 128],
                                                qn[:, dt * 128:(dt + 1) * 128], id32[:])
                        nc.vector.tensor_copy(
                            qT[:, g * 4:(g + 1) * 4, qt * 128:(qt + 1) * 128],
                            tp[:].rearrange("p (j q) -> p j q", j=4))
                ctxf = sb.tile([128, D], fp32, tag="ctxf")
                nc.sync.dma_start(ctxf[:], c_d[b])
                ctx = st3.tile([128, D], fp32r, tag="ctx")
                nc.gpsimd.tensor_copy(ctx[:], ctxf[:])
                ctxT = st3.tile([128, 8, NC], fp32r, tag="ctxT")
                for g in range(2):
                    tp = psp.tile([128, 512], fp32, tag="tr", bufs=2)
                    for j in range(4):
                        dt = g * 4 + j
                        nc.tensor.transpose(tp[:, j * 128:(j + 1) * 128],
                                            ctxf[:, dt * 128:(dt + 1) * 128], id32[:])
                    nc.scalar.copy(ctxT[:, g * 4:(g + 1) * 4],
                                   tp[:].rearrange("p (j c) -> p j c", j=4))

                return qT, ctxf, ctx, ctxT

            def compute(b, st):
                qT, ctxf, ctx, ctxT = st
                mx = st3.tile([128, 8, NQ], fp32, tag="mx")        # matrix^T state
                smooth = st3.tile([128, 2], fp32, tag="smooth")    # per (q%128, qt)

                def scan(i, use_mx, final):
                    pl = psp.tile([128, 512], fp32, tag="ps")
                    if use_mx:
                        qps = []
                        for h in range(2):
                            qp4 = sb.tile([128, 4, NQ], fp32r, tag="qp4")
                            nc.vector.scalar_tensor_tensor(qp4[:], mx[:, h * 4:(h + 1) * 4], 1.0,
                                                           qT[:, h * 4:(h + 1) * 4],
                                                           op0=ALU.min, op1=ALU.mult)
                            qps.append(qp4)
                    for k in range(8):
                        rhs = qps[k // 4][:, k % 4] if use_mx else qT[:, k]
                        nc.tensor.matmul(pl[:, :NQ], ctxT[:, k], rhs,
                                         start=(k == 0), stop=(k == 7))
                    plc = sb.tile([128, NQ], fp32, tag="scr")
                    nc.vector.tensor_copy(plc[:], pl[:, :NQ])
                    lr = sb.tile([128, NQ], fp32, tag="lr")
                    nc.vector.scalar_tensor_tensor(lr[:], plc[:], 0.1, plc[:],
                                                   op0=ALU.mult, op1=ALU.max)
                    scr = sb.tile([128, NQ], fp32, tag="scr")
                    ssq = nrp.tile([128, 1], fp32, tag="ssq")
                    nc.scalar.activation(scr[:], lr[:], AF.Square, accum_out=ssq[:])
                    rs = rsqrt(ssq[:], 1, "a")
                    lrn = sb.tile([128, NQ], fp32r, tag="lrn")
                    nc.vector.tensor_scalar_mul(lrn[:], lr[:], rs[:])
                    pt = psp.tile([128, 512], fp32r, tag="tr", bufs=2)
                    for qt in range(2):
                        nc.tensor.transpose(pt[:, qt * 128:(qt + 1) * 128],
                                            lrn[:, qt * 128:(qt + 1) * 128], idr[:])
                    e = sb.tile([128, 2, NC], fp32, tag="scr")
                    sume = nrp.tile([128, 2], fp32, tag="sume")
                    for qt in range(2):
                        sc = 10.0 if i == 0 else smooth[:, qt:qt + 1]
                        nc.scalar.activation(e[:, qt], pt[:, qt * 128:(qt + 1) * 128],
                                             AF.Exp, scale=sc, accum_out=sume[:, qt:qt + 1])
                    rse = nrp.tile([128, 2], fp32, tag="rse")
                    nc.vector.reciprocal(rse[:], sume[:])
                    p = sb.tile([128, 2, NC], fp32r, tag="p")
                    for qt in range(2):
                        nc.vector.tensor_scalar_mul(p[:, qt], e[:, qt], rse[:, qt:qt + 1])
                    pp = psp.tile([128, 512], fp32r, tag="tr", bufs=2)
                    for qt in range(2):
                        nc.tensor.transpose(pp[:, qt * 128:(qt + 1) * 128], p[:, qt], idr[:])
                    pT = sb.tile([128, NQ], fp32r, tag="pT")
                    nc.vector.tensor_copy(pT[:], pp[:, :NQ])
                    if final:
                        nc.sync.dma_start(at_d[b].rearrange("(t p) c -> p t c", p=128),
                                          p[:].bitcast(fp32))
                        for qt in range(2):
                            for nh in range(2):
                                pw = psp.tile([128, 512], fp32, tag="wc", bufs=2)
                                nc.tensor.matmul(pw[:], pT[:, qt * 128:(qt + 1) * 128],
                                                 ctx[:, nh * 512:(nh + 1) * 512],
                                                 start=True, stop=True)
                                wch = sb.tile([128, 512], fp32, tag="wch")
                                nc.scalar.copy(wch[:], pw[:])
                                nc.sync.dma_start(
                                    wc_d[b, qt * 128:(qt + 1) * 128,
                                         nh * 512:(nh + 1) * 512], wch[:])
                        return None
                    sqTs = []
                    for dt in range(8):
                        pw = psp.tile([128, 512], fp32, tag="wc", bufs=2)
                        nc.tensor.matmul(pw[:, :NQ], ctx[:, dt * 128:(dt + 1) * 128],
                                         pT[:], start=True, stop=True)
                        qmw = sb.tile([128, NQ], fp32, tag="scr")
                        nc.vector.tensor_tensor(qmw[:], qT[:, dt], pw[:, :NQ], op=ALU.subtract)
                        sq = sq9.tile([128, NQ], fp32r, tag="sqT")
                        nc.scalar.activation(sq[:], qmw[:], AF.Square)
                        sqTs.append(sq)
                    return sqTs

                def mlp(i, sqTs):
                    ssqc = nrp.tile([128, 2], fp32, tag="ssqc")
                    pcs = []
                    for _qt in range(2):
                        pc = psp.tile([128, 512], fp32, tag="ps")
                        pcs.append(pc)
                    for k in range(8):
                        for qt in range(2):
                            nc.tensor.matmul(pcs[qt][:, :AD],
                                             sqTs[k][:, qt * 128:(qt + 1) * 128],
                                             cw[:, i, k], start=(k == 0), stop=False)
                    for qt in range(2):
                        nc.tensor.matmul(pcs[qt][:, :AD], ones1r[:], cwb[:, i],
                                         start=False, stop=True)
                        scr2 = sb.tile([128, AD], fp32, tag="scr")
                        nc.scalar.activation(scr2[:], pcs[qt][:, :AD], AF.Square,
                                             accum_out=ssqc[:, qt:qt + 1])
                    rsc = rsqrt(ssqc[:], 2, "c")
                    cn = sb.tile([128, 2, AD], fp32r, tag="cn")
                    for qt in range(2):
                        nc.vector.tensor_scalar_mul(cn[:, qt], pcs[qt][:, :AD], rsc[:, qt:qt + 1])
                    pcT = psp.tile([128, 512], fp32r, tag="tr", bufs=2)
                    for ah in range(2):
                        for qt in range(2):
                            nc.tensor.transpose(pcT[:, ah * 256 + qt * 128:ah * 256 + qt * 128 + 128],
                                                cn[:, qt, ah * 128:(ah + 1) * 128], idr[:])
                    cnT = sb.tile([128, 2, NQ], fp32r, tag="cnT")
                    nc.scalar.copy(cnT[:], pcT[:].rearrange("p (a q) -> p a q", a=2))
                    # matrix branch
                    t1m = sb.tile([128, 4, NQ], fp32r, tag="t1m")
                    for mt in range(4):
                        pm = psp.tile([128, 512], fp32, tag="ps")
                        for ah in range(2):
                            nc.tensor.matmul(pm[:, :NQ], mw1[:, i, ah, mt * 128:(mt + 1) * 128],
                                             cnT[:, ah], start=(ah == 0), stop=(ah == 1))
                        nc.scalar.activation(t1m[:, mt], pm[:, :NQ], AF.Tanh,
                                             bias=mb1[:, i, mt:mt + 1])
                    for dt in range(8):
                        pm2 = psp.tile([128, 512], fp32, tag="ps")
                        for mt in range(4):
                            nc.tensor.matmul(pm2[:, :NQ], mw2[:, i, mt, dt * 128:(dt + 1) * 128],
                                             t1m[:, mt], start=(mt == 0), stop=(mt == 3))
                        th = sb.tile([128, NQ], fp32, tag="lr")
                        nc.scalar.activation(th[:], pm2[:, :NQ], AF.Tanh, bias=mb2[:, i, dt:dt + 1])
                        if i == 0:
                            # matrix stored UNCLIPPED; clip fused into consumers.
                            # clip == min(.,1): value >= -1 always (tanh>=-1, prev>=0)
                            nc.vector.tensor_single_scalar(mx[:, dt], th[:], 1.0, op=ALU.add)
                        else:
                            nc.vector.scalar_tensor_tensor(mx[:, dt], mx[:, dt], 1.0, th[:],
                                                           op0=ALU.min, op1=ALU.add)
                    # smooth branch
                    ps1 = psp.tile([128, 512], fp32, tag="ps")
                    for ah in range(2):
                        nc.tensor.matmul(ps1[:, :NQ], sw1[:, i, ah], cnT[:, ah],
                                         start=(ah == 0), stop=(ah == 1))
                    t1s = sb.tile([128, NQ], fp32, tag="t1s")
                    nc.scalar.activation(t1s[:], ps1[:, :NQ], AF.Tanh, bias=sb1[:, i:i + 1])
                    if i != 0:
                        sm2 = nrp.tile([128, 2], fp32, tag="sm2")
                        nc.vector.tensor_scalar_add(sm2[:], smooth[:], sb2[:, i:i + 1])
                    for qt in range(2):
                        pso = psp.tile([128, 512], fp32, tag="ps")
                        nc.tensor.matmul(pso[:, :1], t1s[:, qt * 128:(qt + 1) * 128],
                                         sw2[:, i:i + 1], start=True, stop=True)
                        ba = sb2p10[:] if i == 0 else sm2[:, qt:qt + 1]
                        nc.scalar.activation(smooth[:, qt:qt + 1], pso[:, :1], AF.Relu, bias=ba)

                # ---- the recurrence ----
                sq0 = scan(0, False, False)
                mlp(0, sq0)
                sq1 = scan(1, True, False)
                mlp(1, sq1)
                scan(2, True, True)

            pending = load(0)
            for b in range(n_b):
                nxt = load(b + 1) if b + 1 < n_b else None
                compute(b, pending)
                pending = nxt

    nc.compile()
    return nc


_NC_CACHE = {}


def _get_nc(n_b):
    if n_b not in _NC_CACHE:
        _NC_CACHE[n_b] = _build(n_b)
    return _NC_CACHE[n_b]


def kernel(query, context, cw_W, cw_b, sw_W1, sw_b1, sw_W2, sw_b2,
           mw_W1, mw_b1, mw_W2, mw_b2, trace=False):
    from concourse import bass_utils

    query = np.ascontiguousarray(np.asarray(query, dtype=np.float32))
    context = np.ascontiguousarray(np.asarray(context, dtype=np.float32))
    w = {k: np.ascontiguousarray(np.asarray(v, dtype=np.float32)) for k, v in dict(
        cw_W=cw_W, cw_b=cw_b, sw_W1=sw_W1, sw_b1=sw_b1, sw_W2=sw_W2, sw_b2=sw_b2,
        mw_W1=mw_W1, mw_b1=mw_b1, mw_W2=mw_W2, mw_b2=mw_b2).items()}

    nc = _get_nc(NB)
    in_maps = []
    for c in range(NCORES):
        sl = slice(c * NB, (c + 1) * NB)
        in_maps.append({"query": query[sl], "context": context[sl], **w})
    res = bass_utils.run_bass_kernel_spmd(nc, in_maps, core_ids=list(range(NCORES)),
                                          trace=trace)
    wc = np.concatenate([res.results[c]["wc_out"] for c in range(NCORES)], axis=0)
    attn = np.concatenate([res.results[c]["attn_out"] for c in range(NCORES)], axis=0)
    kernel.last_results = res
    return query, wc, attn


kernel.last_results = None


# revision 26
# speedup vs baseline: 1.0662x; 1.0029x over previous
"""AttnRCR Trainium2 kernel: 2-layer recurrent cross-attention (B=128, NQ=256, NC=128, D=1024).

Data-parallel over 8 NeuronCores (16 batch elements each). All matmuls in fp32r
(1 cyc/row on PE at N>=256, ~1e-4 accuracy); activations fp32 on ACT (single
`exp_and_others` table: Exp/Tanh/Square/Relu/Copy); rsqrt for the two l2norms
computed on DVE via bit-hack + 2 Newton iterations (no ACT table switch).

Layout strategy: keep activations transposed (feature dim on partitions) so every
matmul contraction lands on partitions without per-scan transposes:
  qT/ctxT (d,q)/(d,c) once per batch via PE transposes; sq=(q-wc)^2 produced
  directly in (d,q); MLP runs in transposed layout where per-feature biases are
  free per-partition ACT bias APs; softmax runs in (q,c) where the per-row smooth
  is a per-partition ACT scale AP.
"""
import numpy as np

B, NQ, NC, D = 128, 256, 128, 1024
AD, NL, SMOOTH0 = 256, 2, 10.0
NCORES = 8
NB = B // NCORES  # batches per core


def _build(n_b):
    import concourse.bacc as bacc
    import concourse.tile as tile
    import concourse.mybir as mybir
    from concourse import masks

    fp32 = mybir.dt.float32
    fp32r = mybir.dt.float32r
    i32 = mybir.dt.int32
    AF = mybir.ActivationFunctionType
    ALU = mybir.AluOpType

    nc = bacc.Bacc("TRN2", debug=False, enable_asserts=False, num_devices=NCORES)

    # ---- DRAM I/O (per-core shard: n_b batches; weights replicated) ----
    q_d = nc.dram_tensor("query", (n_b, NQ, D), fp32, kind="ExternalInput").ap()
    c_d = nc.dram_tensor("context", (n_b, NC, D), fp32, kind="ExternalInput").ap()
    cwW_d = nc.dram_tensor("cw_W", (NL, D, AD), fp32, kind="ExternalInput").ap()
    cwb_d = nc.dram_tensor("cw_b", (NL, AD), fp32, kind="ExternalInput").ap()
    sw1_d = nc.dram_tensor("sw_W1", (NL, AD, AD // 2), fp32, kind="ExternalInput").ap()
    sb1_d = nc.dram_tensor("sw_b1", (NL, AD // 2), fp32, kind="ExternalInput").ap()
    sw2_d = nc.dram_tensor("sw_W2", (NL, AD // 2, 1), fp32, kind="ExternalInput").ap()
    sb2_d = nc.dram_tensor("sw_b2", (NL, 1), fp32, kind="ExternalInput").ap()
    mw1_d = nc.dram_tensor("mw_W1", (NL, AD, 2 * AD), fp32, kind="ExternalInput").ap()
    mb1_d = nc.dram_tensor("mw_b1", (NL, 2 * AD), fp32, kind="ExternalInput").ap()
    mw2_d = nc.dram_tensor("mw_W2", (NL, 2 * AD, D), fp32, kind="ExternalInput").ap()
    mb2_d = nc.dram_tensor("mw_b2", (NL, D), fp32, kind="ExternalInput").ap()
    wc_d = nc.dram_tensor("wc_out", (n_b, NQ, D), fp32, kind="ExternalOutput").ap()
    at_d = nc.dram_tensor("attn_out", (n_b, NQ, NC), fp32, kind="ExternalOutput").ap()

    with tile.TileContext(nc) as tc:
        with tc.tile_pool(name="wp", bufs=1) as wp, \
             tc.tile_pool(name="sb", bufs=2) as sb, \
             tc.tile_pool(name="st3", bufs=3) as st3, \
             tc.tile_pool(name="sq9", bufs=4) as sq9, \
             tc.tile_pool(name="nr", bufs=4) as nrp, \
             tc.tile_pool(name="ps", bufs=4, space="PSUM") as psp:

            # ================= one-time: identity + weights (fp32r) =================
            id32 = wp.tile([128, 128], fp32, tag="id32")
            masks.make_identity(nc, id32[:])
            idr = wp.tile([128, 128], fp32r, tag="idr")
            nc.vector.tensor_copy(idr[:], id32[:])
            ones1 = wp.tile([1, 128], fp32, tag="ones1f")
            nc.vector.memset(ones1[:], 1.0)
            ones1r = wp.tile([1, 128], fp32r, tag="ones1")
            nc.vector.tensor_copy(ones1r[:], ones1[:])

            def load_conv(dst_ap, src_ap, shape, tag):
                st = sb.tile(shape, fp32, tag="qnat")
                nc.sync.dma_start(st[:], src_ap)
                nc.vector.tensor_copy(dst_ap, st[:])

            cw = wp.tile([128, NL, 8, AD], fp32r, tag="cw")
            mw1 = wp.tile([128, NL, 2, 2 * AD], fp32r, tag="mw1")
            mw2 = wp.tile([128, NL, 4, D], fp32r, tag="mw2")
            sw1 = wp.tile([128, NL, 2, AD // 2], fp32r, tag="sw1")
            sw2 = wp.tile([128, NL], fp32, tag="sw2")
            cwb = wp.tile([1, NL, AD], fp32r, tag="cwb")
            for i in range(NL):
                load_conv(cw[:, i], cwW_d[i].rearrange("(k p) a -> p k a", p=128),
                          [128, 8, AD], "wstg")
                load_conv(mw1[:, i], mw1_d[i].rearrange("(k p) m -> p k m", p=128),
                          [128, 2, 2 * AD], "wstg")
                for kk in range(4):
                    load_conv(mw2[:, i, kk], mw2_d[i, kk * 128:(kk + 1) * 128],
                              [128, D], "wstg")
                load_conv(sw1[:, i], sw1_d[i].rearrange("(k p) h -> p k h", p=128),
                          [128, 2, AD // 2], "wstg2")
            load_conv(cwb[:], cwb_d[None, :, :], [1, NL, AD], "wstg4")
            nc.sync.dma_start(sw2[:], sw2_d.rearrange("i p x -> p (i x)"))
            # fp32 per-partition bias tiles
            mb1 = wp.tile([128, NL, 4], fp32, tag="mb1")
            nc.sync.dma_start(mb1[:], mb1_d.rearrange("i (t p) -> p i t", p=128))
            mb2 = wp.tile([128, NL, 8], fp32, tag="mb2")
            nc.sync.dma_start(mb2[:], mb2_d.rearrange("i (t p) -> p i t", p=128))
            sb1 = wp.tile([128, NL], fp32, tag="sb1")
            nc.sync.dma_start(sb1[:], sb1_d.rearrange("i p -> p i"))
            # sw_b2: (NL,1) -> broadcast over partitions via stride-0 DMA read
            sb2 = wp.tile([128, NL], fp32, tag="sb2")
            nc.sync.dma_start(sb2[:], sb2_d.rearrange("i x -> (i x)")[None, :].broadcast_to((128, NL)))
            sb2p10 = wp.tile([128, 1], fp32, tag="sb2p10")
            nc.vector.tensor_single_scalar(sb2p10[:], sb2[:, 0:1], SMOOTH0, op=ALU.add)

            # ================= helpers =================
            def rsqrt(ssq, w, tag):
                """1/sqrt(ssq) on DVE, (128, w) fp32. Bit-hack seed + 2 Newton."""
                y = nrp.tile([128, w], fp32, tag=f"{tag}y")
                u = nrp.tile([128, w], fp32, tag=f"{tag}u")
                m = nrp.tile([128, w], fp32, tag=f"{tag}m")
                yi = y[:].bitcast(i32)
                nc.vector.tensor_single_scalar(yi, ssq.bitcast(i32), 1, op=ALU.arith_shift_right)
                nc.vector.tensor_scalar(yi, yi, -1, 0x5F3759DF, op0=ALU.mult, op1=ALU.add)
                nc.vector.tensor_single_scalar(m[:], ssq, -0.5, op=ALU.mult)
                for _ in range(2):
                    nc.vector.tensor_tensor(u[:], y[:], m[:], op=ALU.mult)
                    nc.vector.tensor_tensor(u[:], u[:], y[:], op=ALU.mult)
                    nc.vector.tensor_single_scalar(u[:], u[:], 1.5, op=ALU.add)
                    nc.vector.tensor_tensor(y[:], y[:], u[:], op=ALU.mult)
                return y

            # ================= per-batch body =================
            def load(b):
                # ---- load + transpose query/context ----
                qT = st3.tile([128, 8, NQ], fp32r, tag="qT")
                for qt in range(2):
                    qn = sb.tile([128, D], fp32, tag="qnat")
                    nc.sync.dma_start(qn[:], q_d[b, qt * 128:(qt + 1) * 128])
                    for g in range(2):  # 4 dt per psum tile
                        tp = psp.tile([128, 512], fp32, tag="tr", bufs=2)
                        for j in range(4):
                            dt = g * 4 + j
                            nc.tensor.transpose(tp[:, j * 128:(j + 1) * 128],
                                                qn[:, dt * 128:(dt + 1) * 128], id32[:])
                        nc.vector.tensor_copy(
                            qT[:, g * 4:(g + 1) * 4, qt * 128:(qt + 1) * 128],
                            tp[:].rearrange("p (j q) -> p j q", j=4))
                ctxf = sb.tile([128, D], fp32, tag="ctxf")
                nc.sync.dma_start(ctxf[:], c_d[b])
                ctx = st3.tile([128, D], fp32r, tag="ctx")
                nc.gpsimd.tensor_copy(ctx[:], ctxf[:])
                ctxT = st3.tile([128, 8, NC], fp32r, tag="ctxT")
                for g in range(2):
                    tp = psp.tile([128, 512], fp32, tag="tr", bufs=2)
                    for j in range(4):
                        dt = g * 4 + j
                        nc.tensor.transpose(tp[:, j * 128:(j + 1) * 128],
                                            ctxf[:, dt * 128:(dt + 1) * 128], id32[:])
                    nc.scalar.copy(ctxT[:, g * 4:(g + 1) * 4],
                                   tp[:].rearrange("p (j c) -> p j c", j=4))

                return qT, ctxf, ctx, ctxT

            def compute(b, st):
                qT, ctxf, ctx, ctxT = st
                mx = st3.tile([128, 8, NQ], fp32, tag="mx")        # matrix^T state
                smooth = st3.tile([128, 2], fp32, tag="smooth")    # per (q%128, qt)

                def scan(i, use_mx, final):
                    pl = psp.tile([128, 512], fp32, tag="ps")
                    if use_mx:
                        qps = []
                        for h in range(2):
                            qp4 = sb.tile([128, 4, NQ], fp32r, tag="qp4")
                            nc.vector.scalar_tensor_tensor(qp4[:], mx[:, h * 4:(h + 1) * 4], 1.0,
                                                           qT[:, h * 4:(h + 1) * 4],
                                                           op0=ALU.min, op1=ALU.mult)
                            qps.append(qp4)
                    for k in range(8):
                        rhs = qps[k // 4][:, k % 4] if use_mx else qT[:, k]
                        nc.tensor.matmul(pl[:, :NQ], ctxT[:, k], rhs,
                                         start=(k == 0), stop=(k == 7))
                    plc = sb.tile([128, NQ], fp32, tag="scr")
                    nc.vector.tensor_copy(plc[:], pl[:, :NQ])
                    lr = sb.tile([128, NQ], fp32, tag="lr")
                    nc.vector.scalar_tensor_tensor(lr[:], plc[:], 0.1, plc[:],
                                                   op0=ALU.mult, op1=ALU.max)
                    scr = sb.tile([128, NQ], fp32, tag="scr")
                    ssq = nrp.tile([128, 1], fp32, tag="ssq")
                    nc.scalar.activation(scr[:], lr[:], AF.Square, accum_out=ssq[:])
                    rs = rsqrt(ssq[:], 1, "a")
                    lrn = sb.tile([128, NQ], fp32r, tag="lrn")
                    nc.vector.tensor_scalar_mul(lrn[:], lr[:], rs[:])
                    pt = psp.tile([128, 512], fp32r, tag="tr", bufs=2)
                    for qt in range(2):
                        nc.tensor.transpose(pt[:, qt * 128:(qt + 1) * 128],
                                            lrn[:, qt * 128:(qt + 1) * 128], idr[:])
                    e = sb.tile([128, 2, NC], fp32, tag="scr")
                    sume = nrp.tile([128, 2], fp32, tag="sume")
                    for qt in range(2):
                        sc = 10.0 if i == 0 else smooth[:, qt:qt + 1]
                        nc.scalar.activation(e[:, qt], pt[:, qt * 128:(qt + 1) * 128],
                                             AF.Exp, scale=sc, accum_out=sume[:, qt:qt + 1])
                    rse = nrp.tile([128, 2], fp32, tag="rse")
                    nc.vector.reciprocal(rse[:], sume[:])
                    p = sb.tile([128, 2, NC], fp32r, tag="p")
                    for qt in range(2):
                        nc.vector.tensor_scalar_mul(p[:, qt], e[:, qt], rse[:, qt:qt + 1])
                    pp = psp.tile([128, 512], fp32r, tag="tr", bufs=2)
                    for qt in range(2):
                        nc.tensor.transpose(pp[:, qt * 128:(qt + 1) * 128], p[:, qt], idr[:])
                    pT = sb.tile([128, NQ], fp32r, tag="pT")
                    nc.vector.tensor_copy(pT[:], pp[:, :NQ])
                    if final:
                        nc.sync.dma_start(at_d[b].rearrange("(t p) c -> p t c", p=128),
                                          p[:].bitcast(fp32))
                        for qt in range(2):
                            for nh in range(2):
                                pw = psp.tile([128, 512], fp32, tag="wc", bufs=2)
                                nc.tensor.matmul(pw[:], pT[:, qt * 128:(qt + 1) * 128],
                                                 ctx[:, nh * 512:(nh + 1) * 512],
                                                 start=True, stop=True)
                                wch = sb.tile([128, 512], fp32, tag="wch")
                                nc.scalar.copy(wch[:], pw[:])
                                nc.sync.dma_start(
                                    wc_d[b, qt * 128:(qt + 1) * 128,
                                         nh * 512:(nh + 1) * 512], wch[:])
                        return None
                    sqTs = []
                    for g in range(4):  # 2 d-tiles packed per psum tile
                        pw = psp.tile([128, 512], fp32, tag="wc", bufs=2)
                        for j in range(2):
                            dt = g * 2 + j
                            nc.tensor.matmul(pw[:, j * NQ:(j + 1) * NQ],
                                             ctx[:, dt * 128:(dt + 1) * 128],
                                             pT[:], start=True, stop=True)
                        qmw = sb.tile([128, 512], fp32, tag="wch")
                        nc.vector.tensor_tensor(qmw[:].rearrange("p (j q) -> p j q", j=2),
                                                qT[:, g * 2:(g + 1) * 2], pw[:],
                                                op=ALU.subtract)
                        sq = sq9.tile([128, 2, NQ], fp32r, tag="sqT")
                        if g % 2 == 0:
                            nc.scalar.activation(sq[:], qmw[:].rearrange("p (j q) -> p j q", j=2),
                                                 AF.Square)
                        else:
                            nc.vector.tensor_tensor(sq[:], qmw[:].rearrange("p (j q) -> p j q", j=2),
                                                    qmw[:].rearrange("p (j q) -> p j q", j=2),
                                                    op=ALU.mult)
                        sqTs.append(sq)
                    return sqTs

                def mlp(i, sqTs):
                    ssqc = nrp.tile([128, 2], fp32, tag="ssqc")
                    pcs = []
                    for _qt in range(2):
                        pc = psp.tile([128, 512], fp32, tag="ps")
                        pcs.append(pc)
                    for k in range(8):
                        for qt in range(2):
                            nc.tensor.matmul(pcs[qt][:, :AD],
                                             sqTs[k // 2][:, k % 2, qt * 128:(qt + 1) * 128],
                                             cw[:, i, k], start=(k == 0), stop=False)
                    for qt in range(2):
                        nc.tensor.matmul(pcs[qt][:, :AD], ones1r[:], cwb[:, i],
                                         start=False, stop=True)
                        scr2 = sb.tile([128, AD], fp32, tag="scr")
                        nc.scalar.activation(scr2[:], pcs[qt][:, :AD], AF.Square,
                                             accum_out=ssqc[:, qt:qt + 1])
                    rsc = rsqrt(ssqc[:], 2, "c")
                    cn = sb.tile([128, 2, AD], fp32r, tag="cn")
                    for qt in range(2):
                        nc.vector.tensor_scalar_mul(cn[:, qt], pcs[qt][:, :AD], rsc[:, qt:qt + 1])
                    pcT = psp.tile([128, 512], fp32r, tag="tr", bufs=2)
                    for ah in range(2):
                        for qt in range(2):
                            nc.tensor.transpose(pcT[:, ah * 256 + qt * 128:ah * 256 + qt * 128 + 128],
                                                cn[:, qt, ah * 128:(ah + 1) * 128], idr[:])
                    cnT = sb.tile([128, 2, NQ], fp32r, tag="cnT")
                    nc.scalar.copy(cnT[:], pcT[:].rearrange("p (a q) -> p a q", a=2))
                    # matrix branch
                    t1m = sb.tile([128, 4, NQ], fp32r, tag="t1m")
                    for mt in range(4):
                        pm = psp.tile([128, 512], fp32, tag="ps")
                        for ah in range(2):
                            nc.tensor.matmul(pm[:, :NQ], mw1[:, i, ah, mt * 128:(mt + 1) * 128],
                                             cnT[:, ah], start=(ah == 0), stop=(ah == 1))
                        nc.scalar.activation(t1m[:, mt], pm[:, :NQ], AF.Tanh,
                                             bias=mb1[:, i, mt:mt + 1])
                    for dt in range(8):
                        pm2 = psp.tile([128, 512], fp32, tag="ps")
                        for mt in range(4):
                            nc.tensor.matmul(pm2[:, :NQ], mw2[:, i, mt, dt * 128:(dt + 1) * 128],
                                             t1m[:, mt], start=(mt == 0), stop=(mt == 3))
                        th = sb.tile([128, NQ], fp32, tag="lr")
                        nc.scalar.activation(th[:], pm2[:, :NQ], AF.Tanh, bias=mb2[:, i, dt:dt + 1])
                        if i == 0:
                            # matrix stored UNCLIPPED; clip fused into consumers.
                            # clip == min(.,1): value >= -1 always (tanh>=-1, prev>=0)
                            nc.vector.tensor_single_scalar(mx[:, dt], th[:], 1.0, op=ALU.add)
                        else:
                            nc.vector.scalar_tensor_tensor(mx[:, dt], mx[:, dt], 1.0, th[:],
                                                           op0=ALU.min, op1=ALU.add)
                    # smooth branch
                    ps1 = psp.tile([128, 512], fp32, tag="ps")
                    for ah in range(2):
                        nc.tensor.matmul(ps1[:, :NQ], sw1[:, i, ah], cnT[:, ah],
                                         start=(ah == 0), stop=(ah == 1))
                    t1s = sb.tile([128, NQ], fp32, tag="t1s")
                    nc.scalar.activation(t1s[:], ps1[:, :NQ], AF.Tanh, bias=sb1[:, i:i + 1])
                    if i != 0:
                        sm2 = nrp.tile([128, 2], fp32, tag="sm2")
                        nc.vector.tensor_scalar_add(sm2[:], smooth[:], sb2[:, i:i + 1])
                    for qt in range(2):
                        pso = psp.tile([128, 512], fp32, tag="ps")
                        nc.tensor.matmul(pso[:, :1], t1s[:, qt * 128:(qt + 1) * 128],
                                         sw2[:, i:i + 1], start=True, stop=True)
                        ba = sb2p10[:] if i == 0 else sm2[:, qt:qt + 1]
                        nc.scalar.activation(smooth[:, qt:qt + 1], pso[:, :1], AF.Relu, bias=ba)

                # ---- the recurrence ----
                sq0 = scan(0, False, False)
                mlp(0, sq0)
                sq1 = scan(1, True, False)
                mlp(1, sq1)
                scan(2, True, True)

            pending = load(0)
            for b in range(n_b):
                nxt = load(b + 1) if b + 1 < n_b else None
                compute(b, pending)
                pending = nxt

    nc.compile()
    return nc


_NC_CACHE = {}


def _get_nc(n_b):
    if n_b not in _NC_CACHE:
        _NC_CACHE[n_b] = _build(n_b)
    return _NC_CACHE[n_b]


def kernel(query, context, cw_W, cw_b, sw_W1, sw_b1, sw_W2, sw_b2,
           mw_W1, mw_b1, mw_W2, mw_b2, trace=False):
    from concourse import bass_utils

    query = np.ascontiguousarray(np.asarray(query, dtype=np.float32))
    context = np.ascontiguousarray(np.asarray(context, dtype=np.float32))
    w = {k: np.ascontiguousarray(np.asarray(v, dtype=np.float32)) for k, v in dict(
        cw_W=cw_W, cw_b=cw_b, sw_W1=sw_W1, sw_b1=sw_b1, sw_W2=sw_W2, sw_b2=sw_b2,
        mw_W1=mw_W1, mw_b1=mw_b1, mw_W2=mw_W2, mw_b2=mw_b2).items()}

    nc = _get_nc(NB)
    in_maps = []
    for c in range(NCORES):
        sl = slice(c * NB, (c + 1) * NB)
        in_maps.append({"query": query[sl], "context": context[sl], **w})
    res = bass_utils.run_bass_kernel_spmd(nc, in_maps, core_ids=list(range(NCORES)),
                                          trace=trace)
    wc = np.concatenate([res.results[c]["wc_out"] for c in range(NCORES)], axis=0)
    attn = np.concatenate([res.results[c]["attn_out"] for c in range(NCORES)], axis=0)
    kernel.last_results = res
    return query, wc, attn


kernel.last_results = None


# revision 30
# speedup vs baseline: 1.1009x; 1.0326x over previous
"""AttnRCR Trainium2 kernel: 2-layer recurrent cross-attention (B=128, NQ=256, NC=128, D=1024).

Data-parallel over 8 NeuronCores (16 batch elements each). All matmuls in fp32r
(1 cyc/row on PE at N>=256, ~1e-4 accuracy); activations fp32 on ACT (single
`exp_and_others` table: Exp/Tanh/Square/Relu/Copy); rsqrt for the two l2norms
computed on DVE via bit-hack + 2 Newton iterations (no ACT table switch).

Layout strategy: keep activations transposed (feature dim on partitions) so every
matmul contraction lands on partitions without per-scan transposes:
  qT/ctxT (d,q)/(d,c) once per batch via PE transposes; sq=(q-wc)^2 produced
  directly in (d,q); MLP runs in transposed layout where per-feature biases are
  free per-partition ACT bias APs; softmax runs in (q,c) where the per-row smooth
  is a per-partition ACT scale AP.
"""
import numpy as np

B, NQ, NC, D = 128, 256, 128, 1024
AD, NL, SMOOTH0 = 256, 2, 10.0
NCORES = 8
NB = B // NCORES  # batches per core


def _build(n_b):
    import concourse.bacc as bacc
    import concourse.tile as tile
    import concourse.mybir as mybir
    from concourse import masks

    fp32 = mybir.dt.float32
    fp32r = mybir.dt.float32r
    i32 = mybir.dt.int32
    AF = mybir.ActivationFunctionType
    ALU = mybir.AluOpType

    nc = bacc.Bacc("TRN2", debug=False, enable_asserts=False, num_devices=NCORES)

    # ---- DRAM I/O (per-core shard: n_b batches; weights replicated) ----
    q_d = nc.dram_tensor("query", (n_b, NQ, D), fp32, kind="ExternalInput").ap()
    c_d = nc.dram_tensor("context", (n_b, NC, D), fp32, kind="ExternalInput").ap()
    cwW_d = nc.dram_tensor("cw_W", (NL, D, AD), fp32, kind="ExternalInput").ap()
    cwb_d = nc.dram_tensor("cw_b", (NL, AD), fp32, kind="ExternalInput").ap()
    sw1_d = nc.dram_tensor("sw_W1", (NL, AD, AD // 2), fp32, kind="ExternalInput").ap()
    sb1_d = nc.dram_tensor("sw_b1", (NL, AD // 2), fp32, kind="ExternalInput").ap()
    sw2_d = nc.dram_tensor("sw_W2", (NL, AD // 2, 1), fp32, kind="ExternalInput").ap()
    sb2_d = nc.dram_tensor("sw_b2", (NL, 1), fp32, kind="ExternalInput").ap()
    mw1_d = nc.dram_tensor("mw_W1", (NL, AD, 2 * AD), fp32, kind="ExternalInput").ap()
    mb1_d = nc.dram_tensor("mw_b1", (NL, 2 * AD), fp32, kind="ExternalInput").ap()
    mw2_d = nc.dram_tensor("mw_W2", (NL, 2 * AD, D), fp32, kind="ExternalInput").ap()
    mb2_d = nc.dram_tensor("mw_b2", (NL, D), fp32, kind="ExternalInput").ap()
    wc_d = nc.dram_tensor("wc_out", (n_b, NQ, D), fp32, kind="ExternalOutput").ap()
    at_d = nc.dram_tensor("attn_out", (n_b, NQ, NC), fp32, kind="ExternalOutput").ap()

    with tile.TileContext(nc) as tc:
        with tc.tile_pool(name="wp", bufs=1) as wp, \
             tc.tile_pool(name="sb", bufs=2) as sb, \
             tc.tile_pool(name="st3", bufs=3) as st3, \
             tc.tile_pool(name="sq9", bufs=4) as sq9, \
             tc.tile_pool(name="nr", bufs=4) as nrp, \
             tc.tile_pool(name="ps", bufs=4, space="PSUM") as psp:

            # ================= one-time: identity + weights (fp32r) =================
            id32 = wp.tile([128, 128], fp32, tag="id32")
            masks.make_identity(nc, id32[:])
            idr = wp.tile([128, 128], fp32r, tag="idr")
            nc.vector.tensor_copy(idr[:], id32[:])
            ones1 = wp.tile([1, 128], fp32, tag="ones1f")
            nc.vector.memset(ones1[:], 1.0)
            ones1r = wp.tile([1, 128], fp32r, tag="ones1")
            nc.vector.tensor_copy(ones1r[:], ones1[:])

            def load_conv(dst_ap, src_ap, shape, tag):
                st = sb.tile(shape, fp32, tag="qnat")
                nc.sync.dma_start(st[:], src_ap)
                nc.vector.tensor_copy(dst_ap, st[:])

            cw = wp.tile([128, NL, 8, AD], fp32r, tag="cw")
            mw1 = wp.tile([128, NL, 2, 2 * AD], fp32r, tag="mw1")
            mw2 = wp.tile([128, NL, 4, D], fp32r, tag="mw2")
            sw1 = wp.tile([128, NL, 2, AD // 2], fp32r, tag="sw1")
            sw2 = wp.tile([128, NL], fp32, tag="sw2")
            cwb = wp.tile([1, NL, AD], fp32r, tag="cwb")
            for i in range(NL):
                load_conv(cw[:, i], cwW_d[i].rearrange("(k p) a -> p k a", p=128),
                          [128, 8, AD], "wstg")
                load_conv(mw1[:, i], mw1_d[i].rearrange("(k p) m -> p k m", p=128),
                          [128, 2, 2 * AD], "wstg")
                for kk in range(4):
                    load_conv(mw2[:, i, kk], mw2_d[i, kk * 128:(kk + 1) * 128],
                              [128, D], "wstg")
                load_conv(sw1[:, i], sw1_d[i].rearrange("(k p) h -> p k h", p=128),
                          [128, 2, AD // 2], "wstg2")
            load_conv(cwb[:], cwb_d[None, :, :], [1, NL, AD], "wstg4")
            nc.sync.dma_start(sw2[:], sw2_d.rearrange("i p x -> p (i x)"))
            # fp32 per-partition bias tiles
            mb1 = wp.tile([128, NL, 4], fp32, tag="mb1")
            nc.sync.dma_start(mb1[:], mb1_d.rearrange("i (t p) -> p i t", p=128))
            mb2 = wp.tile([128, NL, 8], fp32, tag="mb2")
            nc.sync.dma_start(mb2[:], mb2_d.rearrange("i (t p) -> p i t", p=128))
            sb1 = wp.tile([128, NL], fp32, tag="sb1")
            nc.sync.dma_start(sb1[:], sb1_d.rearrange("i p -> p i"))
            # sw_b2: (NL,1) -> broadcast over partitions via stride-0 DMA read
            sb2 = wp.tile([128, NL], fp32, tag="sb2")
            nc.sync.dma_start(sb2[:], sb2_d.rearrange("i x -> (i x)")[None, :].broadcast_to((128, NL)))
            sb2p10 = wp.tile([128, 1], fp32, tag="sb2p10")
            nc.vector.tensor_single_scalar(sb2p10[:], sb2[:, 0:1], SMOOTH0, op=ALU.add)

            # ================= helpers =================
            def rsqrt(ssq, w, tag):
                """1/sqrt(ssq) on DVE, (128, w) fp32. Bit-hack seed + 2 Newton."""
                y = nrp.tile([128, w], fp32, tag=f"{tag}y")
                u = nrp.tile([128, w], fp32, tag=f"{tag}u")
                m = nrp.tile([128, w], fp32, tag=f"{tag}m")
                yi = y[:].bitcast(i32)
                nc.vector.tensor_single_scalar(yi, ssq.bitcast(i32), 1, op=ALU.arith_shift_right)
                nc.vector.tensor_scalar(yi, yi, -1, 0x5F3759DF, op0=ALU.mult, op1=ALU.add)
                nc.vector.tensor_single_scalar(m[:], ssq, -0.5, op=ALU.mult)
                for _ in range(2):
                    nc.vector.tensor_tensor(u[:], y[:], m[:], op=ALU.mult)
                    nc.vector.tensor_tensor(u[:], u[:], y[:], op=ALU.mult)
                    nc.vector.tensor_single_scalar(u[:], u[:], 1.5, op=ALU.add)
                    nc.vector.tensor_tensor(y[:], y[:], u[:], op=ALU.mult)
                return y

            # ================= per-batch body =================
            def load(b):
                # ---- load + transpose query/context ----
                qT = st3.tile([128, 8, NQ], fp32r, tag="qT")
                for qt in range(2):
                    qn = sb.tile([128, D], fp32, tag="qnat")
                    nc.sync.dma_start(qn[:], q_d[b, qt * 128:(qt + 1) * 128])
                    for g in range(2):  # 4 dt per psum tile
                        tp = psp.tile([128, 512], fp32, tag="tr", bufs=2)
                        for j in range(4):
                            dt = g * 4 + j
                            nc.tensor.transpose(tp[:, j * 128:(j + 1) * 128],
                                                qn[:, dt * 128:(dt + 1) * 128], id32[:])
                        nc.vector.tensor_copy(
                            qT[:, g * 4:(g + 1) * 4, qt * 128:(qt + 1) * 128],
                            tp[:].rearrange("p (j q) -> p j q", j=4))
                ctxf = sb.tile([128, D], fp32, tag="ctxf")
                nc.sync.dma_start(ctxf[:], c_d[b])
                ctx = st3.tile([128, D], fp32r, tag="ctx")
                nc.gpsimd.tensor_copy(ctx[:], ctxf[:])
                ctxT = st3.tile([128, 8, NC], fp32r, tag="ctxT")
                for g in range(2):
                    tp = psp.tile([128, 512], fp32, tag="tr", bufs=2)
                    for j in range(4):
                        dt = g * 4 + j
                        nc.tensor.transpose(tp[:, j * 128:(j + 1) * 128],
                                            ctxf[:, dt * 128:(dt + 1) * 128], id32[:])
                    nc.scalar.copy(ctxT[:, g * 4:(g + 1) * 4],
                                   tp[:].rearrange("p (j c) -> p j c", j=4))

                return qT, ctxf, ctx, ctxT

            def compute(b, st):
                qT, ctxf, ctx, ctxT = st
                mx = st3.tile([128, 8, NQ], fp32, tag="mx")        # matrix^T state
                smooth = st3.tile([128, 2], fp32, tag="smooth")    # per (q%128, qt)

                def scan(i, use_mx, final):
                    pl = psp.tile([128, 512], fp32, tag="ps")
                    if use_mx:
                        qps = []
                        for h in range(4):
                            qp4 = sb.tile([128, 2, NQ], fp32r, tag="qp4", bufs=4)
                            nc.vector.scalar_tensor_tensor(qp4[:], mx[:, h * 2:(h + 1) * 2], 1.0,
                                                           qT[:, h * 2:(h + 1) * 2],
                                                           op0=ALU.min, op1=ALU.mult)
                            qps.append(qp4)
                    for k in range(8):
                        rhs = qps[k // 2][:, k % 2] if use_mx else qT[:, k]
                        nc.tensor.matmul(pl[:, :NQ], ctxT[:, k], rhs,
                                         start=(k == 0), stop=(k == 7))
                    plc = sb.tile([128, NQ], fp32, tag="scr")
                    nc.vector.tensor_copy(plc[:], pl[:, :NQ])
                    lr = sb.tile([128, NQ], fp32, tag="lr")
                    nc.vector.scalar_tensor_tensor(lr[:], plc[:], 0.1, plc[:],
                                                   op0=ALU.mult, op1=ALU.max)
                    scr = sb.tile([128, NQ], fp32, tag="scr")
                    ssq = nrp.tile([128, 1], fp32, tag="ssq")
                    nc.scalar.activation(scr[:], lr[:], AF.Square, accum_out=ssq[:])
                    rs = rsqrt(ssq[:], 1, "a")
                    lrn = sb.tile([128, NQ], fp32r, tag="lrn")
                    nc.vector.tensor_scalar_mul(lrn[:], lr[:], rs[:])
                    pt = psp.tile([128, 512], fp32r, tag="tr", bufs=2)
                    for qt in range(2):
                        nc.tensor.transpose(pt[:, qt * 128:(qt + 1) * 128],
                                            lrn[:, qt * 128:(qt + 1) * 128], idr[:])
                    e = sb.tile([128, 2, NC], fp32, tag="scr")
                    sume = nrp.tile([128, 2], fp32, tag="sume")
                    for qt in range(2):
                        sc = 10.0 if i == 0 else smooth[:, qt:qt + 1]
                        nc.scalar.activation(e[:, qt], pt[:, qt * 128:(qt + 1) * 128],
                                             AF.Exp, scale=sc, accum_out=sume[:, qt:qt + 1])
                    rse = nrp.tile([128, 2], fp32, tag="rse")
                    nc.vector.reciprocal(rse[:], sume[:])
                    p = sb.tile([128, 2, NC], fp32r, tag="p")
                    for qt in range(2):
                        nc.vector.tensor_scalar_mul(p[:, qt], e[:, qt], rse[:, qt:qt + 1])
                    pp = psp.tile([128, 512], fp32r, tag="tr", bufs=2)
                    for qt in range(2):
                        nc.tensor.transpose(pp[:, qt * 128:(qt + 1) * 128], p[:, qt], idr[:])
                    pT = sb.tile([128, NQ], fp32r, tag="pT")
                    nc.vector.tensor_copy(pT[:], pp[:, :NQ])
                    if final:
                        nc.sync.dma_start(at_d[b].rearrange("(t p) c -> p t c", p=128),
                                          p[:].bitcast(fp32))
                        for qt in range(2):
                            for nh in range(2):
                                pw = psp.tile([128, 512], fp32, tag="wc", bufs=2)
                                nc.tensor.matmul(pw[:], pT[:, qt * 128:(qt + 1) * 128],
                                                 ctx[:, nh * 512:(nh + 1) * 512],
                                                 start=True, stop=True)
                                wch = sb.tile([128, 512], fp32, tag="wch")
                                nc.scalar.copy(wch[:], pw[:])
                                nc.sync.dma_start(
                                    wc_d[b, qt * 128:(qt + 1) * 128,
                                         nh * 512:(nh + 1) * 512], wch[:])
                        return None
                    sqTs = []
                    for g in range(4):  # 2 d-tiles packed per psum tile
                        pw = psp.tile([128, 512], fp32, tag="wc", bufs=2)
                        for j in range(2):
                            dt = g * 2 + j
                            nc.tensor.matmul(pw[:, j * NQ:(j + 1) * NQ],
                                             ctx[:, dt * 128:(dt + 1) * 128],
                                             pT[:], start=True, stop=True)
                        qmw = sb.tile([128, 512], fp32, tag="wch")
                        nc.vector.tensor_tensor(qmw[:].rearrange("p (j q) -> p j q", j=2),
                                                qT[:, g * 2:(g + 1) * 2], pw[:],
                                                op=ALU.subtract)
                        sq = sq9.tile([128, 2, NQ], fp32r, tag="sqT")
                        if g % 2 == 0:
                            nc.scalar.activation(sq[:], qmw[:].rearrange("p (j q) -> p j q", j=2),
                                                 AF.Square)
                        else:
                            nc.vector.tensor_tensor(sq[:], qmw[:].rearrange("p (j q) -> p j q", j=2),
                                                    qmw[:].rearrange("p (j q) -> p j q", j=2),
                                                    op=ALU.mult)
                        sqTs.append(sq)
                    return sqTs

                def mlp(i, sqTs):
                    ssqc = nrp.tile([128, 2], fp32, tag="ssqc")
                    pcs = []
                    for _qt in range(2):
                        pc = psp.tile([128, 512], fp32, tag="ps")
                        pcs.append(pc)
                    for k in range(8):
                        for qt in range(2):
                            nc.tensor.matmul(pcs[qt][:, :AD],
                                             sqTs[k // 2][:, k % 2, qt * 128:(qt + 1) * 128],
                                             cw[:, i, k], start=(k == 0), stop=False)
                    for qt in range(2):
                        nc.tensor.matmul(pcs[qt][:, :AD], ones1r[:], cwb[:, i],
                                         start=False, stop=True)
                        scr2 = sb.tile([128, AD], fp32, tag="scr")
                        nc.scalar.activation(scr2[:], pcs[qt][:, :AD], AF.Square,
                                             accum_out=ssqc[:, qt:qt + 1])
                    rsc = rsqrt(ssqc[:], 2, "c")
                    cn = sb.tile([128, 2, AD], fp32r, tag="cn")
                    for qt in range(2):
                        nc.vector.tensor_scalar_mul(cn[:, qt], pcs[qt][:, :AD], rsc[:, qt:qt + 1])
                    pcT = psp.tile([128, 512], fp32r, tag="tr", bufs=2)
                    for ah in range(2):
                        for qt in range(2):
                            nc.tensor.transpose(pcT[:, ah * 256 + qt * 128:ah * 256 + qt * 128 + 128],
                                                cn[:, qt, ah * 128:(ah + 1) * 128], idr[:])
                    cnT = sb.tile([128, 2, NQ], fp32r, tag="cnT")
                    nc.scalar.copy(cnT[:], pcT[:].rearrange("p (a q) -> p a q", a=2))
                    # matrix branch
                    t1m = sb.tile([128, 4, NQ], fp32r, tag="t1m")
                    for mt in range(4):
                        pm = psp.tile([128, 512], fp32, tag="ps")
                        for ah in range(2):
                            nc.tensor.matmul(pm[:, :NQ], mw1[:, i, ah, mt * 128:(mt + 1) * 128],
                                             cnT[:, ah], start=(ah == 0), stop=(ah == 1))
                        nc.scalar.activation(t1m[:, mt], pm[:, :NQ], AF.Tanh,
                                             bias=mb1[:, i, mt:mt + 1])
                    for dt in range(8):
                        pm2 = psp.tile([128, 512], fp32, tag="ps")
                        for mt in range(4):
                            nc.tensor.matmul(pm2[:, :NQ], mw2[:, i, mt, dt * 128:(dt + 1) * 128],
                                             t1m[:, mt], start=(mt == 0), stop=(mt == 3))
                        th = sb.tile([128, NQ], fp32, tag="lr")
                        nc.scalar.activation(th[:], pm2[:, :NQ], AF.Tanh, bias=mb2[:, i, dt:dt + 1])
                        if i == 0:
                            # matrix stored UNCLIPPED; clip fused into consumers.
                            # clip == min(.,1): value >= -1 always (tanh>=-1, prev>=0)
                            nc.vector.tensor_single_scalar(mx[:, dt], th[:], 1.0, op=ALU.add)
                        else:
                            nc.vector.scalar_tensor_tensor(mx[:, dt], mx[:, dt], 1.0, th[:],
                                                           op0=ALU.min, op1=ALU.add)
                    # smooth branch
                    ps1 = psp.tile([128, 512], fp32, tag="ps")
                    for ah in range(2):
                        nc.tensor.matmul(ps1[:, :NQ], sw1[:, i, ah], cnT[:, ah],
                                         start=(ah == 0), stop=(ah == 1))
                    t1s = sb.tile([128, NQ], fp32, tag="t1s")
                    nc.scalar.activation(t1s[:], ps1[:, :NQ], AF.Tanh, bias=sb1[:, i:i + 1])
                    if i != 0:
                        sm2 = nrp.tile([128, 2], fp32, tag="sm2")
                        nc.vector.tensor_scalar_add(sm2[:], smooth[:], sb2[:, i:i + 1])
                    for qt in range(2):
                        pso = psp.tile([128, 512], fp32, tag="ps")
                        nc.tensor.matmul(pso[:, :1], t1s[:, qt * 128:(qt + 1) * 128],
                                         sw2[:, i:i + 1], start=True, stop=True)
                        ba = sb2p10[:] if i == 0 else sm2[:, qt:qt + 1]
                        nc.scalar.activation(smooth[:, qt:qt + 1], pso[:, :1], AF.Relu, bias=ba)

                # ---- the recurrence ----
                sq0 = scan(0, False, False)
                mlp(0, sq0)
                sq1 = scan(1, True, False)
                mlp(1, sq1)
                scan(2, True, True)

            pending = load(0)
            for b in range(n_b):
                nxt = load(b + 1) if b + 1 < n_b else None
                compute(b, pending)
                pending = nxt

    nc.compile()
    return nc


_NC_CACHE = {}


def _get_nc(n_b):
    if n_b not in _NC_CACHE:
        _NC_CACHE[n_b] = _build(n_b)
    return _NC_CACHE[n_b]


def kernel(query, context, cw_W, cw_b, sw_W1, sw_b1, sw_W2, sw_b2,
           mw_W1, mw_b1, mw_W2, mw_b2, trace=False):
    from concourse import bass_utils

    query = np.ascontiguousarray(np.asarray(query, dtype=np.float32))
    context = np.ascontiguousarray(np.asarray(context, dtype=np.float32))
    w = {k: np.ascontiguousarray(np.asarray(v, dtype=np.float32)) for k, v in dict(
        cw_W=cw_W, cw_b=cw_b, sw_W1=sw_W1, sw_b1=sw_b1, sw_W2=sw_W2, sw_b2=sw_b2,
        mw_W1=mw_W1, mw_b1=mw_b1, mw_W2=mw_W2, mw_b2=mw_b2).items()}

    nc = _get_nc(NB)
    in_maps = []
    for c in range(NCORES):
        sl = slice(c * NB, (c + 1) * NB)
        in_maps.append({"query": query[sl], "context": context[sl], **w})
    res = bass_utils.run_bass_kernel_spmd(nc, in_maps, core_ids=list(range(NCORES)),
                                          trace=trace)
    wc = np.concatenate([res.results[c]["wc_out"] for c in range(NCORES)], axis=0)
    attn = np.concatenate([res.results[c]["attn_out"] for c in range(NCORES)], axis=0)
    kernel.last_results = res
    return query, wc, attn


kernel.last_results = None


# revision 37
# speedup vs baseline: 1.1114x; 1.0095x over previous
"""AttnRCR Trainium2 kernel: 2-layer recurrent cross-attention (B=128, NQ=256, NC=128, D=1024).

Data-parallel over 8 NeuronCores (16 batch elements each). All matmuls in fp32r
(1 cyc/row on PE at N>=256, ~1e-4 accuracy); activations fp32 on ACT (single
`exp_and_others` table: Exp/Tanh/Square/Relu/Copy); rsqrt for the two l2norms
computed on DVE via bit-hack + 2 Newton iterations (no ACT table switch).

Layout strategy: keep activations transposed (feature dim on partitions) so every
matmul contraction lands on partitions without per-scan transposes:
  qT/ctxT (d,q)/(d,c) once per batch via PE transposes; sq=(q-wc)^2 produced
  directly in (d,q); MLP runs in transposed layout where per-feature biases are
  free per-partition ACT bias APs; softmax runs in (q,c) where the per-row smooth
  is a per-partition ACT scale AP.
"""
import numpy as np

B, NQ, NC, D = 128, 256, 128, 1024
AD, NL, SMOOTH0 = 256, 2, 10.0
NCORES = 8
NB = B // NCORES  # batches per core


def _build(n_b):
    import concourse.bacc as bacc
    import concourse.tile as tile
    import concourse.mybir as mybir
    from concourse import masks

    fp32 = mybir.dt.float32
    fp32r = mybir.dt.float32r
    i32 = mybir.dt.int32
    AF = mybir.ActivationFunctionType
    ALU = mybir.AluOpType

    nc = bacc.Bacc("TRN2", debug=False, enable_asserts=False, num_devices=NCORES)

    # ---- DRAM I/O (per-core shard: n_b batches; weights replicated) ----
    q_d = nc.dram_tensor("query", (n_b, NQ, D), fp32, kind="ExternalInput").ap()
    c_d = nc.dram_tensor("context", (n_b, NC, D), fp32, kind="ExternalInput").ap()
    cwW_d = nc.dram_tensor("cw_W", (NL, D, AD), fp32, kind="ExternalInput").ap()
    cwb_d = nc.dram_tensor("cw_b", (NL, AD), fp32, kind="ExternalInput").ap()
    sw1_d = nc.dram_tensor("sw_W1", (NL, AD, AD // 2), fp32, kind="ExternalInput").ap()
    sb1_d = nc.dram_tensor("sw_b1", (NL, AD // 2), fp32, kind="ExternalInput").ap()
    sw2_d = nc.dram_tensor("sw_W2", (NL, AD // 2, 1), fp32, kind="ExternalInput").ap()
    sb2_d = nc.dram_tensor("sw_b2", (NL, 1), fp32, kind="ExternalInput").ap()
    mw1_d = nc.dram_tensor("mw_W1", (NL, AD, 2 * AD), fp32, kind="ExternalInput").ap()
    mb1_d = nc.dram_tensor("mw_b1", (NL, 2 * AD), fp32, kind="ExternalInput").ap()
    mw2_d = nc.dram_tensor("mw_W2", (NL, 2 * AD, D), fp32, kind="ExternalInput").ap()
    mb2_d = nc.dram_tensor("mw_b2", (NL, D), fp32, kind="ExternalInput").ap()
    wc_d = nc.dram_tensor("wc_out", (n_b, NQ, D), fp32, kind="ExternalOutput").ap()
    at_d = nc.dram_tensor("attn_out", (n_b, NQ, NC), fp32, kind="ExternalOutput").ap()

    with tile.TileContext(nc) as tc:
        with tc.tile_pool(name="wp", bufs=1) as wp, \
             tc.tile_pool(name="sb", bufs=2) as sb, \
             tc.tile_pool(name="st3", bufs=3) as st3, \
             tc.tile_pool(name="sq9", bufs=4) as sq9, \
             tc.tile_pool(name="nr", bufs=4) as nrp, \
             tc.tile_pool(name="ps", bufs=4, space="PSUM") as psp:

            # ================= one-time: identity + weights (fp32r) =================
            id32 = wp.tile([128, 128], fp32, tag="id32")
            masks.make_identity(nc, id32[:])
            idr = wp.tile([128, 128], fp32r, tag="idr")
            nc.vector.tensor_copy(idr[:], id32[:])
            ones1 = wp.tile([1, 128], fp32, tag="ones1f")
            nc.vector.memset(ones1[:], 1.0)
            ones1r = wp.tile([1, 128], fp32r, tag="ones1")
            nc.vector.tensor_copy(ones1r[:], ones1[:])

            def load_conv(dst_ap, src_ap, shape, tag):
                st = sb.tile(shape, fp32, tag="qnat")
                nc.sync.dma_start(st[:], src_ap)
                nc.vector.tensor_copy(dst_ap, st[:])

            cw = wp.tile([128, NL, 8, AD], fp32r, tag="cw")
            mw1 = wp.tile([128, NL, 2, 2 * AD], fp32r, tag="mw1")
            mw2 = wp.tile([128, NL, 4, D], fp32r, tag="mw2")
            sw1 = wp.tile([128, NL, 2, AD // 2], fp32r, tag="sw1")
            sw2 = wp.tile([128, NL], fp32, tag="sw2")
            cwb = wp.tile([1, NL, AD], fp32r, tag="cwb")
            for i in range(NL):
                load_conv(cw[:, i], cwW_d[i].rearrange("(k p) a -> p k a", p=128),
                          [128, 8, AD], "wstg")
                load_conv(mw1[:, i], mw1_d[i].rearrange("(k p) m -> p k m", p=128),
                          [128, 2, 2 * AD], "wstg")
                for kk in range(4):
                    load_conv(mw2[:, i, kk], mw2_d[i, kk * 128:(kk + 1) * 128],
                              [128, D], "wstg")
                load_conv(sw1[:, i], sw1_d[i].rearrange("(k p) h -> p k h", p=128),
                          [128, 2, AD // 2], "wstg2")
            load_conv(cwb[:], cwb_d[None, :, :], [1, NL, AD], "wstg4")
            nc.sync.dma_start(sw2[:], sw2_d.rearrange("i p x -> p (i x)"))
            # fp32 per-partition bias tiles
            mb1 = wp.tile([128, NL, 4], fp32, tag="mb1")
            nc.sync.dma_start(mb1[:], mb1_d.rearrange("i (t p) -> p i t", p=128))
            mb2 = wp.tile([128, NL, 8], fp32, tag="mb2")
            nc.sync.dma_start(mb2[:], mb2_d.rearrange("i (t p) -> p i t", p=128))
            sb1 = wp.tile([128, NL], fp32, tag="sb1")
            nc.sync.dma_start(sb1[:], sb1_d.rearrange("i p -> p i"))
            # sw_b2: (NL,1) -> broadcast over partitions via stride-0 DMA read
            sb2 = wp.tile([128, NL], fp32, tag="sb2")
            nc.sync.dma_start(sb2[:], sb2_d.rearrange("i x -> (i x)")[None, :].broadcast_to((128, NL)))
            sb2p10 = wp.tile([128, 1], fp32, tag="sb2p10")
            nc.vector.tensor_single_scalar(sb2p10[:], sb2[:, 0:1], SMOOTH0, op=ALU.add)

            # ================= helpers =================
            def rsqrt(ssq, w, tag):
                """1/sqrt(ssq) on DVE, (128, w) fp32. Bit-hack seed + 2 Newton."""
                y = nrp.tile([128, w], fp32, tag=f"{tag}y")
                u = nrp.tile([128, w], fp32, tag=f"{tag}u")
                m = nrp.tile([128, w], fp32, tag=f"{tag}m")
                yi = y[:].bitcast(i32)
                nc.vector.tensor_single_scalar(yi, ssq.bitcast(i32), 1, op=ALU.arith_shift_right)
                nc.vector.tensor_scalar(yi, yi, -1, 0x5F3759DF, op0=ALU.mult, op1=ALU.add)
                nc.vector.tensor_single_scalar(m[:], ssq, -0.5, op=ALU.mult)
                for _ in range(2):
                    nc.vector.tensor_tensor(u[:], y[:], m[:], op=ALU.mult)
                    nc.vector.tensor_tensor(u[:], u[:], y[:], op=ALU.mult)
                    nc.vector.tensor_single_scalar(u[:], u[:], 1.5, op=ALU.add)
                    nc.vector.tensor_tensor(y[:], y[:], u[:], op=ALU.mult)
                return y

            # ================= per-batch body =================
            def load(b):
                # ---- load + transpose query/context ----
                qT = st3.tile([128, 8, NQ], fp32r, tag="qT")
                for qt in range(2):
                    qn = sb.tile([128, D], fp32, tag="qnat")
                    nc.sync.dma_start(qn[:], q_d[b, qt * 128:(qt + 1) * 128])
                    for g in range(2):  # 4 dt per psum tile
                        tp = psp.tile([128, 512], fp32, tag="tr", bufs=2)
                        for j in range(4):
                            dt = g * 4 + j
                            nc.tensor.transpose(tp[:, j * 128:(j + 1) * 128],
                                                qn[:, dt * 128:(dt + 1) * 128], id32[:])
                        eng = nc.vector.tensor_copy if g == 0 else nc.scalar.copy
                        eng(qT[:, g * 4:(g + 1) * 4, qt * 128:(qt + 1) * 128],
                            tp[:].rearrange("p (j q) -> p j q", j=4))
                ctxf = sb.tile([128, D], fp32, tag="ctxf")
                nc.sync.dma_start(ctxf[:], c_d[b])
                ctx = st3.tile([128, D], fp32r, tag="ctx")
                nc.gpsimd.tensor_copy(ctx[:], ctxf[:])
                ctxT = st3.tile([128, 8, NC], fp32r, tag="ctxT")
                for g in range(2):
                    tp = psp.tile([128, 512], fp32, tag="tr", bufs=2)
                    for j in range(4):
                        dt = g * 4 + j
                        nc.tensor.transpose(tp[:, j * 128:(j + 1) * 128],
                                            ctxf[:, dt * 128:(dt + 1) * 128], id32[:])
                    nc.scalar.copy(ctxT[:, g * 4:(g + 1) * 4],
                                   tp[:].rearrange("p (j c) -> p j c", j=4))

                return qT, ctxf, ctx, ctxT

            def compute(b, st):
                qT, ctxf, ctx, ctxT = st
                mx = st3.tile([128, 8, NQ], fp32, tag="mx")        # matrix^T state
                smooth = st3.tile([128, 2], fp32, tag="smooth")    # per (q%128, qt)

                def scan(i, use_mx, final):
                    pl = psp.tile([128, 512], fp32, tag="ps")
                    if use_mx:
                        qps = []
                        for h in range(4):
                            qp4 = sb.tile([128, 2, NQ], fp32r, tag="qp4", bufs=4)
                            nc.vector.scalar_tensor_tensor(qp4[:], mx[:, h * 2:(h + 1) * 2], 1.0,
                                                           qT[:, h * 2:(h + 1) * 2],
                                                           op0=ALU.min, op1=ALU.mult)
                            qps.append(qp4)
                    for k in range(8):
                        rhs = qps[k // 2][:, k % 2] if use_mx else qT[:, k]
                        nc.tensor.matmul(pl[:, :NQ], ctxT[:, k], rhs,
                                         start=(k == 0), stop=(k == 7))
                    plc = sb.tile([128, NQ], fp32, tag="scr")
                    nc.vector.tensor_copy(plc[:], pl[:, :NQ])
                    lr = sb.tile([128, NQ], fp32, tag="lr")
                    nc.vector.scalar_tensor_tensor(lr[:], plc[:], 0.1, plc[:],
                                                   op0=ALU.mult, op1=ALU.max)
                    scr = sb.tile([128, NQ], fp32, tag="scr")
                    ssq = nrp.tile([128, 1], fp32, tag="ssq")
                    nc.scalar.activation(scr[:], lr[:], AF.Square, accum_out=ssq[:])
                    rs = rsqrt(ssq[:], 1, "a")
                    lrn = sb.tile([128, NQ], fp32r, tag="lrn")
                    nc.vector.tensor_scalar_mul(lrn[:], lr[:], rs[:])
                    pt = psp.tile([128, 512], fp32r, tag="tr", bufs=2)
                    for qt in range(2):
                        nc.tensor.transpose(pt[:, qt * 128:(qt + 1) * 128],
                                            lrn[:, qt * 128:(qt + 1) * 128], idr[:])
                    e = sb.tile([128, 2, NC], fp32, tag="scr")
                    sume = nrp.tile([128, 2], fp32, tag="sume")
                    for qt in range(2):
                        sc = 10.0 if i == 0 else smooth[:, qt:qt + 1]
                        nc.scalar.activation(e[:, qt], pt[:, qt * 128:(qt + 1) * 128],
                                             AF.Exp, scale=sc, accum_out=sume[:, qt:qt + 1])
                    rse = nrp.tile([128, 2], fp32, tag="rse")
                    nc.vector.reciprocal(rse[:], sume[:])
                    p = sb.tile([128, 2, NC], fp32r, tag="p")
                    for qt in range(2):
                        nc.vector.tensor_scalar_mul(p[:, qt], e[:, qt], rse[:, qt:qt + 1])
                    pp = psp.tile([128, 512], fp32r, tag="tr", bufs=2)
                    for qt in range(2):
                        nc.tensor.transpose(pp[:, qt * 128:(qt + 1) * 128], p[:, qt], idr[:])
                    pT = sb.tile([128, NQ], fp32r, tag="pT")
                    nc.vector.tensor_copy(pT[:], pp[:, :NQ])
                    if final:
                        nc.sync.dma_start(at_d[b].rearrange("(t p) c -> p t c", p=128),
                                          p[:].bitcast(fp32))
                        for qt in range(2):
                            for nh in range(2):
                                pw = psp.tile([128, 512], fp32, tag="wc", bufs=2)
                                nc.tensor.matmul(pw[:], pT[:, qt * 128:(qt + 1) * 128],
                                                 ctx[:, nh * 512:(nh + 1) * 512],
                                                 start=True, stop=True)
                                wch = sb.tile([128, 512], fp32, tag="wch")
                                nc.scalar.copy(wch[:], pw[:])
                                nc.sync.dma_start(
                                    wc_d[b, qt * 128:(qt + 1) * 128,
                                         nh * 512:(nh + 1) * 512], wch[:])
                        return None
                    sqTs = []
                    for g in range(4):  # 2 d-tiles packed per psum tile
                        pw = psp.tile([128, 512], fp32, tag="wc", bufs=2)
                        for j in range(2):
                            dt = g * 2 + j
                            nc.tensor.matmul(pw[:, j * NQ:(j + 1) * NQ],
                                             ctx[:, dt * 128:(dt + 1) * 128],
                                             pT[:], start=True, stop=True)
                        qmw = sb.tile([128, 512], fp32, tag="wch")
                        nc.vector.tensor_tensor(qmw[:].rearrange("p (j q) -> p j q", j=2),
                                                qT[:, g * 2:(g + 1) * 2], pw[:],
                                                op=ALU.subtract)
                        sq = sq9.tile([128, 2, NQ], fp32r, tag="sqT")
                        if g % 2 == 0:
                            nc.scalar.activation(sq[:], qmw[:].rearrange("p (j q) -> p j q", j=2),
                                                 AF.Square)
                        else:
                            nc.vector.tensor_tensor(sq[:], qmw[:].rearrange("p (j q) -> p j q", j=2),
                                                    qmw[:].rearrange("p (j q) -> p j q", j=2),
                                                    op=ALU.mult)
                        sqTs.append(sq)
                    return sqTs

                def mlp(i, sqTs):
                    ssqc = nrp.tile([128, 2], fp32, tag="ssqc")
                    pcs = []
                    for _qt in range(2):
                        pc = psp.tile([128, 512], fp32, tag="ps")
                        pcs.append(pc)
                    for k in range(8):
                        for qt in range(2):
                            nc.tensor.matmul(pcs[qt][:, :AD],
                                             sqTs[k // 2][:, k % 2, qt * 128:(qt + 1) * 128],
                                             cw[:, i, k], start=(k == 0), stop=False)
                    for qt in range(2):
                        nc.tensor.matmul(pcs[qt][:, :AD], ones1r[:], cwb[:, i],
                                         start=False, stop=True)
                        scr2 = sb.tile([128, AD], fp32, tag="scr")
                        nc.scalar.activation(scr2[:], pcs[qt][:, :AD], AF.Square,
                                             accum_out=ssqc[:, qt:qt + 1])
                    rsc = rsqrt(ssqc[:], 2, "c")
                    cn = sb.tile([128, 2, AD], fp32r, tag="cn")
                    for qt in range(2):
                        nc.vector.tensor_scalar_mul(cn[:, qt], pcs[qt][:, :AD], rsc[:, qt:qt + 1])
                    pcT = psp.tile([128, 512], fp32r, tag="tr", bufs=2)
                    for ah in range(2):
                        for qt in range(2):
                            nc.tensor.transpose(pcT[:, ah * 256 + qt * 128:ah * 256 + qt * 128 + 128],
                                                cn[:, qt, ah * 128:(ah + 1) * 128], idr[:])
                    cnT = sb.tile([128, 2, NQ], fp32r, tag="cnT")
                    nc.scalar.copy(cnT[:], pcT[:].rearrange("p (a q) -> p a q", a=2))
                    # matrix branch
                    t1m = sb.tile([128, 4, NQ], fp32r, tag="t1m")
                    for mt in range(4):
                        pm = psp.tile([128, 512], fp32, tag="ps")
                        for ah in range(2):
                            nc.tensor.matmul(pm[:, :NQ], mw1[:, i, ah, mt * 128:(mt + 1) * 128],
                                             cnT[:, ah], start=(ah == 0), stop=(ah == 1))
                        nc.scalar.activation(t1m[:, mt], pm[:, :NQ], AF.Tanh,
                                             bias=mb1[:, i, mt:mt + 1])
                    for dt in range(8):
                        pm2 = psp.tile([128, 512], fp32, tag="ps")
                        for mt in range(4):
                            nc.tensor.matmul(pm2[:, :NQ], mw2[:, i, mt, dt * 128:(dt + 1) * 128],
                                             t1m[:, mt], start=(mt == 0), stop=(mt == 3))
                        th = sb.tile([128, NQ], fp32, tag="lr")
                        nc.scalar.activation(th[:], pm2[:, :NQ], AF.Tanh, bias=mb2[:, i, dt:dt + 1])
                        if i == 0:
                            # matrix stored UNCLIPPED; clip fused into consumers.
                            # clip == min(.,1): value >= -1 always (tanh>=-1, prev>=0)
                            nc.vector.tensor_single_scalar(mx[:, dt], th[:], 1.0, op=ALU.add)
                        else:
                            nc.vector.scalar_tensor_tensor(mx[:, dt], mx[:, dt], 1.0, th[:],
                                                           op0=ALU.min, op1=ALU.add)
                    # smooth branch
                    ps1 = psp.tile([128, 512], fp32, tag="ps")
                    for ah in range(2):
                        nc.tensor.matmul(ps1[:, :NQ], sw1[:, i, ah], cnT[:, ah],
                                         start=(ah == 0), stop=(ah == 1))
                    t1s = sb.tile([128, NQ], fp32, tag="t1s")
                    nc.scalar.activation(t1s[:], ps1[:, :NQ], AF.Tanh, bias=sb1[:, i:i + 1])
                    if i != 0:
                        sm2 = nrp.tile([128, 2], fp32, tag="sm2")
                        nc.vector.tensor_scalar_add(sm2[:], smooth[:], sb2[:, i:i + 1])
                    for qt in range(2):
                        pso = psp.tile([128, 512], fp32, tag="ps")
                        nc.tensor.matmul(pso[:, :1], t1s[:, qt * 128:(qt + 1) * 128],
                                         sw2[:, i:i + 1], start=True, stop=True)
                        ba = sb2p10[:] if i == 0 else sm2[:, qt:qt + 1]
                        nc.scalar.activation(smooth[:, qt:qt + 1], pso[:, :1], AF.Relu, bias=ba)

                # ---- the recurrence ----
                sq0 = scan(0, False, False)
                mlp(0, sq0)
                sq1 = scan(1, True, False)
                mlp(1, sq1)
                scan(2, True, True)

            pending = load(0)
            for b in range(n_b):
                nxt = load(b + 1) if b + 1 < n_b else None
                compute(b, pending)
                pending = nxt

    nc.compile()
    return nc


_NC_CACHE = {}


def _get_nc(n_b):
    if n_b not in _NC_CACHE:
        _NC_CACHE[n_b] = _build(n_b)
    return _NC_CACHE[n_b]


def kernel(query, context, cw_W, cw_b, sw_W1, sw_b1, sw_W2, sw_b2,
           mw_W1, mw_b1, mw_W2, mw_b2, trace=False):
    from concourse import bass_utils

    query = np.ascontiguousarray(np.asarray(query, dtype=np.float32))
    context = np.ascontiguousarray(np.asarray(context, dtype=np.float32))
    w = {k: np.ascontiguousarray(np.asarray(v, dtype=np.float32)) for k, v in dict(
        cw_W=cw_W, cw_b=cw_b, sw_W1=sw_W1, sw_b1=sw_b1, sw_W2=sw_W2, sw_b2=sw_b2,
        mw_W1=mw_W1, mw_b1=mw_b1, mw_W2=mw_W2, mw_b2=mw_b2).items()}

    nc = _get_nc(NB)
    in_maps = []
    for c in range(NCORES):
        sl = slice(c * NB, (c + 1) * NB)
        in_maps.append({"query": query[sl], "context": context[sl], **w})
    res = bass_utils.run_bass_kernel_spmd(nc, in_maps, core_ids=list(range(NCORES)),
                                          trace=trace)
    wc = np.concatenate([res.results[c]["wc_out"] for c in range(NCORES)], axis=0)
    attn = np.concatenate([res.results[c]["attn_out"] for c in range(NCORES)], axis=0)
    kernel.last_results = res
    return query, wc, attn


kernel.last_results = None
